# revision 31
# baseline (speedup 1.0000x reference)
"""PointNet++ FeaturePropagation Trainium2 kernel (8-core SPMD).

Per core c of 8: batch b = c//2, query-half h = c%2 (8192 original points).
KNN scores are computed on the PE as a 27-row bf16-triple decomposition of
2*(q-c_t).(s-c_t) - |s-c_t|^2 per query tile (center c_t), which makes every
product exact and keeps fp32 PSUM accumulation error ~1e-7 -- required
because 3rd/4th-neighbor distance gaps go down to ~1e-7 on this data.
Queries are Hilbert-sorted on the host; each 128-query tile scores against
a contiguous window of W=512 Hilbert-sorted samples (provably containing
the true 3-NN via a cheap host-side 3rd-NN upper bound); outlier queries
that need wider windows go to 3 dense tiles scored against all 4096
samples.  DVE max/max_index extract top-8; inverse-distance weights and
the gather/interp/MLP/BatchNorm pipeline follow (BN stats via 8-core
AllReduce; conv biases cancel through BN and are skipped).
Host does layout transforms (sorting/transpose/decomposition) and the
final unshard/unpermute.
"""

import numpy as np
import ml_dtypes

import concourse.bass as bass
import concourse.bacc as bacc
import concourse.mybir as mybir
import concourse.tile as tile

F32 = mybir.dt.float32
BF16 = mybir.dt.bfloat16
U32 = mybir.dt.uint32
ALU = mybir.AluOpType
ACT = mybir.ActivationFunctionType
AX = mybir.AxisListType

B, S, N = 4, 4096, 16384
CS, CO = 256, 128
C1, C2 = 256, 256
NCORES = 8
QP = N // 2          # real queries per core
N_EASY, N_HARD, W_EASY = 62, 3, 512
NT = N_EASY + N_HARD  # 65 tiles -> 8320 slots (128 duplicate pads)
BN_EPS = 1e-5
W_EPS = 1e-8
KROWS = 27


def divisor_csz(nq):
    for d in range(512, 0, -1):
        if nq % d == 0:
            return d


def build_program(n_easy=N_EASY, n_hard=N_HARD, w_easy=W_EASY, s_total=S,
                  n_cores=NCORES, n_points_total=None):
    nt = n_easy + n_hard
    nq = nt * 128
    if n_points_total is None:
        n_points_total = n_cores * nq
    nc = bacc.Bacc("TRN2", target_bir_lowering=False, debug=False,
                   num_devices=n_cores)

    d_lhsT = nc.dram_tensor("lhsT", [KROWS, nt, 128], BF16,
                            kind="ExternalInput")
    d_rhsE = nc.dram_tensor("rhsE", [KROWS, max(n_easy, 1), w_easy], BF16,
                            kind="ExternalInput")
    d_rhsH = nc.dram_tensor("rhsH", [KROWS, max(n_hard, 1), s_total], BF16,
                            kind="ExternalInput")
    d_xn23 = nc.dram_tensor("xn23", [128, nt, 3], F32, kind="ExternalInput")
    d_af3 = nc.dram_tensor("af3", [128, nt, 3], F32, kind="ExternalInput")
    d_sfeat = nc.dram_tensor("sfeat", [s_total, CS], F32,
                             kind="ExternalInput")
    d_ofT = nc.dram_tensor("ofT", [CO, nq], BF16, kind="ExternalInput")
    d_w0T = nc.dram_tensor("w0T", [128, 3, C1], BF16, kind="ExternalInput")
    d_w1T = nc.dram_tensor("w1T", [128, 2, C2], BF16, kind="ExternalInput")
    d_g0 = nc.dram_tensor("g0p", [128, 2], F32, kind="ExternalInput")
    d_bt0 = nc.dram_tensor("bt0p", [128, 2], F32, kind="ExternalInput")
    d_g1 = nc.dram_tensor("g1p", [128, 2], F32, kind="ExternalInput")
    d_bt1 = nc.dram_tensor("bt1p", [128, 2], F32, kind="ExternalInput")
    d_eye = nc.dram_tensor("eye", [128, 128], F32, kind="ExternalInput")
    d_out = nc.dram_tensor("yT", [2, 128, nq], F32, kind="ExternalOutput")
    d_dbgi = nc.dram_tensor("dbgi", [128, nt, 3], F32, kind="ExternalOutput")
    d_dbgw = nc.dram_tensor("dbgw", [128, nt, 3], F32, kind="ExternalOutput")

    gs = 2
    with tile.TileContext(nc) as tc:
        with (
            tc.tile_pool(name="const", bufs=1) as cpool,
            tc.tile_pool(name="big", bufs=1) as bigp,
            tc.tile_pool(name="sc_sbE", bufs=2) as scpE,
            tc.tile_pool(name="sc_sbH", bufs=1) as scpH,
            tc.tile_pool(name="small", bufs=3) as smp,
            tc.tile_pool(name="gath", bufs=1) as gathp,
            tc.tile_pool(name="ps_sc", bufs=4, space="PSUM") as ps_sc,
            tc.tile_pool(name="ps_sm", bufs=2, space="PSUM") as ps_sm,
            tc.tile_pool(name="ps_mm1", bufs=2, space="PSUM") as ps_mm1,
            tc.tile_pool(name="dram", bufs=1, space="DRAM") as dramp,
        ):
            def load(pool, name, dram, shape, dt=F32):
                t_ = pool.tile(shape, dt, tag=name)
                nc.sync.dma_start(t_[:], dram[:])
                return t_

            eye = load(cpool, "eye", d_eye, [128, 128])
            w0T = load(cpool, "w0T", d_w0T, [128, 3, C1], BF16)
            w1T = load(cpool, "w1T", d_w1T, [128, 2, C2], BF16)
            g0p = load(cpool, "g0p", d_g0, [128, 2])
            bt0p = load(cpool, "bt0p", d_bt0, [128, 2])
            g1p = load(cpool, "g1p", d_g1, [128, 2])
            bt1p = load(cpool, "bt1p", d_bt1, [128, 2])
            xn23 = load(cpool, "xn23", d_xn23, [128, nt, 3])
            af3 = load(cpool, "af3", d_af3, [128, nt, 3])
            lhsT = load(cpool, "lhsT", d_lhsT, [KROWS, nt, 128], BF16)

            y0a = bigp.tile([128, nq], F32, tag="y0a")
            y0b = bigp.tile([128, nq], F32, tag="y0b")
            idx_all = bigp.tile([128, nt, 3], F32, tag="idx_all")
            w_all = bigp.tile([128, nt, 3], F32, tag="w_all")
            v8a = bigp.tile([128, nt, 8], F32, tag="v8a")
            i8a = bigp.tile([128, nt, 8], U32, tag="i8a")

            # ---------------- KNN selection ----------------
            for t in range(nt):
                if t < n_easy:
                    wt = w_easy
                    rt = scpE.tile([KROWS, w_easy], BF16, tag="rhsEt")
                    nc.sync.dma_start(rt[:], d_rhsE[:, t, :])
                    sc = ps_sc.tile([128, 512], F32, tag="ps_score")
                    nc.tensor.matmul(sc[:, 0:wt], lhsT[:, t, :],
                                     rt[:], start=True, stop=True)
                else:
                    wt = s_total
                    j = t - n_easy
                    rt = scpH.tile([KROWS, s_total], BF16, tag="rhsHt")
                    nc.sync.dma_start(rt[:], d_rhsH[:, j, :])
                    sc = scpH.tile([128, s_total], F32, tag="scoresH")
                    for q8 in range(s_total // 512):
                        ps = ps_sc.tile([128, 512], F32, tag="ps_score")
                        col = q8 * 512
                        nc.tensor.matmul(ps[:], lhsT[:, t, :],
                                         rt[:, col:col + 512],
                                         start=True, stop=True)
                        nc.scalar.copy(sc[:, col:col + 512], ps[:])

                nc.vector.max(v8a[:, t, :], sc[:, 0:wt])
                nc.vector.max_index(i8a[:, t, :], v8a[:, t, :], sc[:, 0:wt])

            # ---------------- batched weights + index remap ----------------
            i8f = bigp.tile([128, nt, 3], F32, tag="i8f")
            nc.vector.tensor_copy(i8f[:], i8a[:, :, 0:3])
            nc.vector.tensor_tensor(out=idx_all[:], in0=i8f[:], in1=af3[:],
                                    op=ALU.add)
            d3a = bigp.tile([128, nt, 3], F32, tag="d3a")
            nc.vector.tensor_tensor(out=d3a[:], in0=xn23[:],
                                    in1=v8a[:, :, 0:3], op=ALU.subtract)
            nc.vector.tensor_scalar_add(d3a[:], d3a[:], W_EPS)
            r3a = bigp.tile([128, nt, 3], F32, tag="r3a")
            nc.vector.reciprocal(r3a[:], d3a[:])
            rsa = bigp.tile([128, nt], F32, tag="rsa")
            nc.vector.tensor_reduce(out=rsa[:], in_=r3a[:], axis=AX.X,
                                    op=ALU.add)
            nc.vector.reciprocal(rsa[:], rsa[:])
            for kk in range(3):
                nc.vector.tensor_tensor(out=w_all[:, :, kk], in0=r3a[:, :, kk],
                                        in1=rsa[:], op=ALU.mult)

            nc.sync.dma_start(d_dbgi[:], idx_all[:])
            nc.sync.dma_start(d_dbgw[:], w_all[:])

            # ---------------- gather + interp + matmul0 ----------------
            # build 16-partition-wrapped int16 index tensor for dma_gather:
            # flat order i = t*384 + k*128 + q -> [i%16, i//16]; since
            # 384%16==128%16==0, partition = q%16 and the col layout is
            # [t*24 + k*8 + q//16] -> 8 strided DMAs (one per q//16 block).
            idx16 = bigp.tile([128, nt, 3], mybir.dt.int16, tag="idx16")
            nc.vector.tensor_copy(idx16[:], idx_all[:])
            ncols = nt * 24
            wrapped = bigp.tile([128, ncols], mybir.dt.int16, tag="wrapped")
            for u in range(8):
                nc.gpsimd.dma_start(
                    wrapped[0:16, u:ncols:8].rearrange(
                        "p (t k) -> p t k", k=3),
                    idx16[16 * u:16 * (u + 1), :, :])
            for rep in range(1, 8):
                nc.gpsimd.dma_start(wrapped[16 * rep:16 * (rep + 1), :],
                                    wrapped[0:16, :])
            n_g = (nt + gs - 1) // gs
            for g in range(n_g):
                t0 = g * gs
                gt = min(gs, nt - t0)
                gbuf = gathp.tile([128, gs * 3, CS], F32, tag="gath")
                ofTg = smp.tile([CO, gs * 128], BF16, tag="ofTg")
                nc.sync.dma_start(ofTg[:, 0:gt * 128],
                                  d_ofT[:, t0 * 128:(t0 + gt) * 128])
                nc.gpsimd.dma_gather(
                    out_ap=gbuf[:, 0:gt * 3, :],
                    in_ap=d_sfeat[:],
                    idxs_ap=wrapped[:, t0 * 24:(t0 + gt) * 24],
                    num_idxs=gt * 384,
                    num_idxs_reg=gt * 384,
                    elem_size=CS,
                )
                for tt in range(gt):
                    t = t0 + tt
                    interp = smp.tile([128, CS], F32, tag="interp")
                    acc = smp.tile([128, CS], F32, tag="interp_acc")
                    nc.vector.tensor_scalar(
                        out=acc[:], in0=gbuf[:, tt * 3, :],
                        scalar1=w_all[:, t, 0:1], scalar2=None, op0=ALU.mult)
                    nc.vector.scalar_tensor_tensor(
                        out=interp[:], in0=gbuf[:, tt * 3 + 1, :],
                        scalar=w_all[:, t, 1:2], in1=acc[:],
                        op0=ALU.mult, op1=ALU.add)
                    nc.vector.scalar_tensor_tensor(
                        out=interp[:], in0=gbuf[:, tt * 3 + 2, :],
                        scalar=w_all[:, t, 2:3], in1=interp[:],
                        op0=ALU.mult, op1=ALU.add)
                    iT = smp.tile([128, 2, 128], BF16, tag="interpT")
                    for hh in range(2):
                        ps_tr = ps_sm.tile([128, 128], F32, tag="ps_small")
                        nc.tensor.transpose(
                            ps_tr[:], interp[:, hh * 128:(hh + 1) * 128],
                            eye[:])
                        nc.vector.tensor_copy(iT[:, hh, :], ps_tr[:])
                    for m, ybuf in ((0, y0a), (1, y0b)):
                        ps_y = ps_sm.tile([128, 128], F32, tag="ps_small")
                        mcol = slice(m * 128, (m + 1) * 128)
                        nc.tensor.matmul(ps_y[:], w0T[:, 0, mcol],
                                         ofTg[:, tt * 128:(tt + 1) * 128],
                                         start=True, stop=False)
                        nc.tensor.matmul(ps_y[:], w0T[:, 1, mcol],
                                         iT[:, 0, :], start=False, stop=False)
                        nc.tensor.matmul(ps_y[:], w0T[:, 2, mcol],
                                         iT[:, 1, :], start=False, stop=True)
                        nc.scalar.copy(ybuf[:, t * 128:(t + 1) * 128], ps_y[:])

            # ---------------- BN helpers ----------------
            csz = divisor_csz(nq)

            def bn_allreduce(ya, yb, gp, btp, tag):
                """Returns (a, bhat) [128,2] with yhat = Relu(y*a + bhat)."""
                nchunk = nq // csz
                sump = smp.tile([128, 2 * nchunk], F32, tag=f"sump{tag}")
                sqp = smp.tile([128, 2 * nchunk], F32, tag=f"sqp{tag}")
                scratch = cpool.tile([128, csz], F32, tag="bn_scratch")
                for m, ybuf in ((0, ya), (1, yb)):
                    for ch in range(nchunk):
                        sl = slice(ch * csz, (ch + 1) * csz)
                        col = m * nchunk + ch
                        nc.vector.tensor_reduce(
                            out=sump[:, col:col + 1], in_=ybuf[:, sl],
                            axis=AX.X, op=ALU.add)
                        nc.scalar.activation(
                            out=scratch[:], in_=ybuf[:, sl], func=ACT.Square,
                            accum_out=sqp[:, col:col + 1])
                stats = smp.tile([128, 4], F32, tag=f"stats{tag}")
                nc.vector.tensor_reduce(
                    out=stats[:, 0:2],
                    in_=sump[:].rearrange("p (m c) -> p m c", m=2),
                    axis=AX.X, op=ALU.add)
                nc.vector.tensor_reduce(
                    out=stats[:, 2:4],
                    in_=sqp[:].rearrange("p (m c) -> p m c", m=2),
                    axis=AX.X, op=ALU.add)
                bi = dramp.tile([128, 4], F32, tag=f"bi{tag}")
                bo = dramp.tile([128, 4], F32, tag=f"bo{tag}")
                nc.gpsimd.dma_start(bi[:], stats[:])
                nc.gpsimd.collective_compute(
                    "AllReduce", ALU.add,
                    replica_groups=[list(range(n_cores))],
                    ins=[bi.opt()], outs=[bo.opt()])
                gstats = smp.tile([128, 4], F32, tag=f"gstats{tag}")
                nc.gpsimd.dma_start(gstats[:], bo[:])

                mean = smp.tile([128, 2], F32, tag=f"mean{tag}")
                nc.vector.tensor_scalar_mul(mean[:], gstats[:, 0:2],
                                            1.0 / n_points_total)
                vpe = smp.tile([128, 2], F32, tag=f"vpe{tag}")
                nc.vector.tensor_scalar_mul(vpe[:], gstats[:, 2:4],
                                            1.0 / n_points_total)
                msq = smp.tile([128, 2], F32, tag=f"msq{tag}")
                nc.vector.tensor_tensor(out=msq[:], in0=mean[:], in1=mean[:],
                                        op=ALU.mult)
                nc.vector.tensor_tensor(out=vpe[:], in0=vpe[:], in1=msq[:],
                                        op=ALU.subtract)
                nc.vector.tensor_scalar_add(vpe[:], vpe[:], BN_EPS)
                rcp = smp.tile([128, 2], F32, tag=f"rcp{tag}")
                nc.vector.reciprocal(rcp[:], vpe[:])
                rsq = smp.tile([128, 2], F32, tag=f"rsq{tag}")
                nc.scalar.activation(out=rsq[:], in_=rcp[:], func=ACT.Sqrt)
                t1 = smp.tile([128, 2], F32, tag=f"t1{tag}")
                nc.vector.tensor_tensor(out=t1[:], in0=rsq[:], in1=rsq[:],
                                        op=ALU.mult)
                nc.vector.tensor_tensor(out=t1[:], in0=t1[:], in1=vpe[:],
                                        op=ALU.mult)
                nc.vector.tensor_scalar(out=t1[:], in0=t1[:], scalar1=-0.5,
                                        scalar2=1.5, op0=ALU.mult, op1=ALU.add)
                nc.vector.tensor_tensor(out=rsq[:], in0=rsq[:], in1=t1[:],
                                        op=ALU.mult)
                a = smp.tile([128, 2], F32, tag=f"a{tag}")
                nc.vector.tensor_tensor(out=a[:], in0=gp[:], in1=rsq[:],
                                        op=ALU.mult)
                bhat = smp.tile([128, 2], F32, tag=f"bhat{tag}")
                nc.vector.tensor_tensor(out=bhat[:], in0=mean[:], in1=a[:],
                                        op=ALU.mult)
                nc.vector.tensor_tensor(out=bhat[:], in0=btp[:], in1=bhat[:],
                                        op=ALU.subtract)
                return a, bhat

            # ---------------- layer 1 ----------------
            a0, b0h = bn_allreduce(y0a, y0b, g0p, bt0p, "0")
            for ch in range(nq // csz):
                sl = slice(ch * csz, (ch + 1) * csz)
                yh0 = smp.tile([128, csz], BF16, tag="yh0")
                yh1 = smp.tile([128, csz], BF16, tag="yh1")
                nc.scalar.activation(out=yh0[:], in_=y0a[:, sl], func=ACT.Relu,
                                     scale=a0[:, 0:1], bias=b0h[:, 0:1])
                nc.scalar.activation(out=yh1[:], in_=y0b[:, sl], func=ACT.Relu,
                                     scale=a0[:, 1:2], bias=b0h[:, 1:2])
                for m, ybuf in ((0, y0a), (1, y0b)):
                    ps1 = ps_mm1.tile([128, csz], F32, tag="ps_mm1")
                    mcol = slice(m * 128, (m + 1) * 128)
                    nc.tensor.matmul(ps1[:], w1T[:, 0, mcol], yh0[:],
                                     start=True, stop=False)
                    nc.tensor.matmul(ps1[:], w1T[:, 1, mcol], yh1[:],
                                     start=False, stop=True)
                    nc.scalar.copy(ybuf[:, sl], ps1[:])

            # ---------------- layer 2 BN + out ----------------
            a1, b1h = bn_allreduce(y0a, y0b, g1p, bt1p, "1")
            for ch in range(nq // csz):
                sl = slice(ch * csz, (ch + 1) * csz)
                for m, ybuf in ((0, y0a), (1, y0b)):
                    o = smp.tile([128, csz], F32, tag="outsb")
                    nc.scalar.activation(out=o[:], in_=ybuf[:, sl],
                                         func=ACT.Relu,
                                         scale=a1[:, m:m + 1],
                                         bias=b1h[:, m:m + 1])
                    nc.sync.dma_start(d_out[m, :, sl], o[:])

    nc.compile()
    return nc


# ======================= host-side preparation =======================

def _hilbert_d3(x, y, z, order=10):
    X = np.stack([x, y, z], axis=0).astype(np.uint32).copy()
    M = np.uint32(1 << (order - 1))
    Q = M
    while Q > 1:
        P = np.uint32(Q - 1)
        for i in range(3):
            cond = (X[i] & Q) != 0
            X[0] = np.where(cond, X[0] ^ P, X[0])
            t = (X[0] ^ X[i]) & P
            X[0] ^= t
            X[i] ^= t
        Q >>= 1
    for i in range(1, 3):
        X[i] ^= X[i - 1]
    t2 = np.zeros_like(X[0])
    Q = M
    while Q > 1:
        t2 = np.where((X[2] & Q) != 0, t2 ^ np.uint32(Q - 1), t2)
        Q >>= 1
    for i in range(3):
        X[i] ^= t2
    key = np.zeros(X.shape[1], dtype=np.uint64)
    for b in range(order - 1, -1, -1):
        for i in range(3):
            key = ((key << np.uint64(1))
                   | ((X[i] >> np.uint32(b)) & np.uint32(1)).astype(np.uint64))
    return key


def _hkeys(pts, lo, hi, order=10):
    qq = ((pts - lo) / np.maximum(hi - lo, 1e-9)).clip(0.0, 1.0)
    qq = (qq * ((1 << order) - 1)).astype(np.uint32)
    return _hilbert_d3(qq[:, 0], qq[:, 1], qq[:, 2], order)


def _bf16(x):
    return np.asarray(np.asarray(x, np.float32), ml_dtypes.bfloat16)


def _dec3(x):
    """fp64 -> three bf16 terms (residual ~2^-24 |x|)."""
    h = _bf16(x)
    r = x - np.asarray(h, np.float64)
    m = _bf16(r)
    r2 = r - np.asarray(m, np.float64)
    return h, m, _bf16(r2)


def _score_rows(u, v):
    """u: [nq, 3] fp64 query offsets; v: [w, 3] fp64 sample offsets.
    Returns L [27, nq] bf16, R [27, w] bf16 with sum_k L[k] outer R[k]
    ~= 2 u.v - |v|^2 (error ~1e-7 for |u|,|v| <~ 1)."""
    uh, um, ul = _dec3(2.0 * u)
    vh, vm, vl = _dec3(v)
    n = v * v
    n1, n2, n3 = _dec3(-n)
    one = np.ones(u.shape[0], ml_dtypes.bfloat16)
    L, R = [], []
    for ci in range(3):       # hi block: products ~O(r^2), cancel early
        L += [uh[:, ci], one]
        R += [vh[:, ci], n1[:, ci]]
    for ci in range(3):       # med block
        L += [um[:, ci], uh[:, ci], one]
        R += [vh[:, ci], vm[:, ci], n2[:, ci]]
    for ci in range(3):       # lo block
        L += [ul[:, ci], uh[:, ci], um[:, ci], one]
        R += [vh[:, ci], vl[:, ci], vm[:, ci], n3[:, ci]]
    return (np.ascontiguousarray(np.stack(L)),
            np.ascontiguousarray(np.stack(R)))


def _pack_core(q, ss, glo, ghi, w_easy, n_easy, n_hard, s_total):
    """q: [nq_real, 3] fp64 queries (this core); ss: [s_total, 3] fp64
    samples sorted by Hilbert key on the (glo, ghi) grid.  Returns (perm,
    a_t) where perm is the slot -> local-query-index map (len nt*128, with
    duplicate pads) and a_t the per-easy-tile window starts."""
    nq_real = q.shape[0]
    nt = n_easy + n_hard
    slots = nt * 128
    e_slots, h_slots = n_easy * 128, n_hard * 128

    qk = _hkeys(q.astype(np.float32), glo, ghi)
    sk = _hkeys(ss.astype(np.float32), glo, ghi)
    oq = np.argsort(qk, kind="stable")
    # ss must already be sorted by its key for searchsorted windows
    qs = q[oq]
    pos = np.searchsorted(np.sort(sk), qk[oq])
    ncand = min(48, s_total)
    start = (pos - ncand // 2).clip(0, s_total - ncand)
    cand = start[:, None] + np.arange(ncand)[None, :]
    dc = ((qs[:, None, :] - ss[cand]) ** 2).sum(-1)
    r3sq = np.sort(dc, 1)[:, 2] * (1 + 1e-5) + 2e-6
    # window per query from full distance matrix (fp32 blas, with margin)
    qf, sf = qs.astype(np.float32), ss.astype(np.float32)
    D = ((qf ** 2).sum(-1)[:, None] + (sf ** 2).sum(-1)[None, :]
         - 2.0 * (qf @ sf.T)).astype(np.float64)
    within = D <= (r3sq[:, None] + 4e-6)
    first = np.argmax(within, 1)
    last = s_total - 1 - np.argmax(within[:, ::-1], 1)
    wq = last - first + 1

    easy = [i for i in range(nq_real) if wq[i] <= w_easy]
    hard = [i for i in range(nq_real) if wq[i] > w_easy]

    def greedy(lst):
        """Pack hilbert-ordered queries into tiles with union <= w_easy."""
        tiles, cur, lo, hi = [], [], None, None
        for i in lst:
            nlo = first[i] if lo is None else min(lo, first[i])
            nhi = last[i] if hi is None else max(hi, last[i])
            if len(cur) < 128 and nhi - nlo + 1 <= w_easy:
                cur.append(i)
                lo, hi = nlo, nhi
            else:
                tiles.append(cur)
                cur, lo, hi = [i], first[i], last[i]
        if cur:
            tiles.append(cur)
        return tiles

    for _ in range(400):
        if len(easy) > e_slots:
            drop = set(sorted(easy, key=lambda i: wq[i])[e_slots:])
            hard += [i for i in easy if i in drop]
            easy = [i for i in easy if i not in drop]
        tiles = greedy(easy)
        if len(tiles) <= n_easy:
            break
        drop = set(sorted(easy, key=lambda i: wq[i])[-8:])
        hard += [i for i in easy if i in drop]
        easy = [i for i in easy if i not in drop]
    if len(hard) > h_slots:
        # degraded fallback (windows of forced-back queries get clamped);
        # should not trigger on sane inputs
        back = set(sorted(hard, key=lambda i: wq[i])[:len(hard) - h_slots])
        easy = sorted(easy + [i for i in hard if i in back],
                      key=lambda i: int(np.searchsorted(np.sort(qk), qk[oq[i]])))
        hard = [i for i in hard if i not in back]
        tiles = greedy(easy)[:n_easy]
        placed = set()
        for tl in tiles:
            placed |= set(tl)
        hard += [i for i in easy if i not in placed]
        easy = [i for i in easy if i in placed]
        hard = hard[:h_slots]

    def pad128(tl):
        # cyclic duplication spreads pad weight evenly over the tile's
        # members (pads are counted in the BN-stat population)
        base = list(tl)
        j = 0
        while len(tl) < 128:
            tl.append(base[j % len(base)])
            j += 1
        return tl

    tiles = [pad128(tl) for tl in tiles]
    while len(tiles) < n_easy:
        tiles.append(list(tiles[-1]))
    a_t = []
    for tl in tiles:
        lo = min(first[i] for i in tl)
        a_t.append(int(min(lo, s_total - w_easy)))

    hard = sorted(hard, key=lambda i: wq[i])
    while len(hard) < h_slots:
        hard.append((len(hard) * 97 + 13) % nq_real)

    flat_easy = [i for tl in tiles for i in tl]
    perm = np.array([oq[i] for i in flat_easy] + [oq[i] for i in hard])
    assert perm.shape[0] == slots
    return perm, a_t


_PROGRAM_CACHE = {}


def make_core_inputs(sampled_xyz, sampled_features, original_xyz,
                     original_features, w0, w1, g0, bt0, g1, bt1, core,
                     n_easy=N_EASY, n_hard=N_HARD, w_easy=W_EASY):
    b, h = core // 2, core % 2
    s_total = sampled_xyz.shape[1]
    nq_real = original_xyz.shape[1] // 2
    nt = n_easy + n_hard
    nq = nt * 128
    f32 = np.float32

    s64 = np.asarray(sampled_xyz[b], np.float64)
    allp = np.vstack([np.asarray(original_xyz[b], f32),
                      np.asarray(sampled_xyz[b], f32)])
    glo, ghi = allp.min(0), allp.max(0)
    os_ = np.argsort(_hkeys(s64.astype(f32), glo, ghi), kind="stable")
    ss = s64[os_]
    sfeat = np.ascontiguousarray(
        np.asarray(sampled_features[b], f32)[os_])

    q = np.asarray(original_xyz[b, h * nq_real:(h + 1) * nq_real], np.float64)
    of = np.asarray(original_features[b, h * nq_real:(h + 1) * nq_real], f32)

    perm, a_t = _pack_core(q, ss, glo, ghi, w_easy, n_easy, n_hard, s_total)
    a_t = list(a_t) + [0] * n_hard

    lhsT = np.zeros((KROWS, nt, 128), ml_dtypes.bfloat16)
    rhsE = np.zeros((KROWS, max(n_easy, 1), w_easy), ml_dtypes.bfloat16)
    rhsH = np.zeros((KROWS, max(n_hard, 1), s_total), ml_dtypes.bfloat16)
    xn2p = np.zeros((128, nt), f32)  # expanded to xn23 below
    for t in range(nt):
        ql = perm[t * 128:(t + 1) * 128]
        qt = q[ql]
        c = qt.mean(0)
        u = qt - c
        if t < n_easy:
            v = ss[a_t[t]:a_t[t] + w_easy] - c
            L, R = _score_rows(u, v)
            rhsE[:, t, :] = R
        else:
            v = ss - c
            L, R = _score_rows(u, v)
            rhsH[:, t - n_easy, :] = R
        lhsT[:, t, :] = L
        xn2p[:, t] = (u * u).sum(-1).astype(f32)

    of_perm = of[perm]                                   # [nq, CO]
    af = np.repeat(np.asarray(a_t, f32)[None, :], 128, 0)
    return {
        "lhsT": lhsT,
        "rhsE": rhsE,
        "rhsH": rhsH,
        "xn23": np.ascontiguousarray(
            np.repeat(xn2p[:, :, None], 3, axis=2)),
        "af3": np.ascontiguousarray(
            np.repeat(af[:, :, None], 3, axis=2)),
        "sfeat": sfeat,
        "ofT": _bf16(np.ascontiguousarray(of_perm.T)),
        "w0T": _bf16(np.ascontiguousarray(
            w0.T.reshape(3, 128, C1).transpose(1, 0, 2))),
        "w1T": _bf16(np.ascontiguousarray(
            w1.T.reshape(2, 128, C2).transpose(1, 0, 2))),
        "g0p": np.ascontiguousarray(g0.reshape(2, 128).T).astype(f32),
        "bt0p": np.ascontiguousarray(bt0.reshape(2, 128).T).astype(f32),
        "g1p": np.ascontiguousarray(g1.reshape(2, 128).T).astype(f32),
        "bt1p": np.ascontiguousarray(bt1.reshape(2, 128).T).astype(f32),
        "eye": np.eye(128, dtype=f32),
    }, perm


def kernel(sampled_xyz, sampled_features, original_xyz, original_features,
           w0, b0, g0, bt0, w1, b1, g1, bt1, k):
    assert int(k) == 3
    from concourse.bass_utils import run_bass_kernel_spmd

    key = "full"
    if key not in _PROGRAM_CACHE:
        _PROGRAM_CACHE[key] = build_program()
    nc = _PROGRAM_CACHE[key]

    args = (sampled_xyz, sampled_features, original_xyz, original_features,
            w0, w1, g0, bt0, g1, bt1)
    args = [np.asarray(a, np.float32) for a in args]
    in_maps, perms = [], []
    for c in range(NCORES):
        im, perm = make_core_inputs(*args, core=c)
        in_maps.append(im)
        perms.append(perm)
    res = run_bass_kernel_spmd(nc, in_maps, core_ids=list(range(NCORES)))
    out = np.empty((B, N, C2), np.float32)
    nq = NT * 128
    for c in range(NCORES):
        b, h = c // 2, c % 2
        yT = res.results[c]["yT"]            # [2, 128, nq]
        y = yT.reshape(256, nq).T            # [nq, 256]
        inv = np.zeros(QP, np.int64)
        inv[perms[c][::-1]] = np.arange(nq)[::-1]  # first occurrence wins
        out[b, h * QP:(h + 1) * QP] = y[inv]
    return out


# revision 33
# speedup vs baseline: 1.0759x; 1.0759x over previous
"""PointNet++ FeaturePropagation Trainium2 kernel (8-core SPMD).

Per core c of 8: batch b = c//2, query-half h = c%2 (8192 original points).
KNN scores are computed on the PE as a 27-row bf16-triple decomposition of
2*(q-c_t).(s-c_t) - |s-c_t|^2 per query tile (center c_t), which makes every
product exact and keeps fp32 PSUM accumulation error ~1e-7 -- required
because 3rd/4th-neighbor distance gaps go down to ~1e-7 on this data.
Queries are Hilbert-sorted on the host; each 128-query tile scores against
a contiguous window of W=512 Hilbert-sorted samples (provably containing
the true 3-NN via a cheap host-side 3rd-NN upper bound); outlier queries
that need wider windows go to 3 dense tiles scored against all 4096
samples.  DVE max/max_index extract top-8; inverse-distance weights and
the gather/interp/MLP/BatchNorm pipeline follow (BN stats via 8-core
AllReduce; conv biases cancel through BN and are skipped).
Host does layout transforms (sorting/transpose/decomposition) and the
final unshard/unpermute.
"""

import numpy as np
import ml_dtypes

import concourse.bass as bass
import concourse.bacc as bacc
import concourse.mybir as mybir
import concourse.tile as tile

F32 = mybir.dt.float32
BF16 = mybir.dt.bfloat16
U32 = mybir.dt.uint32
ALU = mybir.AluOpType
ACT = mybir.ActivationFunctionType
AX = mybir.AxisListType

B, S, N = 4, 4096, 16384
CS, CO = 256, 128
C1, C2 = 256, 256
NCORES = 8
QP = N // 2          # real queries per core
N_EASY, N_HARD, W_EASY = 62, 3, 512
NT = N_EASY + N_HARD  # 65 tiles -> 8320 slots (128 duplicate pads)
BN_EPS = 1e-5
W_EPS = 1e-8
KROWS = 27


def divisor_csz(nq):
    for d in range(512, 0, -1):
        if nq % d == 0:
            return d


def build_program(n_easy=N_EASY, n_hard=N_HARD, w_easy=W_EASY, s_total=S,
                  n_cores=NCORES, n_points_total=None):
    nt = n_easy + n_hard
    nq = nt * 128
    if n_points_total is None:
        n_points_total = n_cores * nq
    nc = bacc.Bacc("TRN2", target_bir_lowering=False, debug=False,
                   num_devices=n_cores)

    d_lhsT = nc.dram_tensor("lhsT", [KROWS, nt, 128], BF16,
                            kind="ExternalInput")
    d_rhsE = nc.dram_tensor("rhsE", [KROWS, max(n_easy, 1), w_easy], BF16,
                            kind="ExternalInput")
    d_rhsH = nc.dram_tensor("rhsH", [KROWS, max(n_hard, 1), s_total], BF16,
                            kind="ExternalInput")
    d_xn23 = nc.dram_tensor("xn23", [128, nt, 3], F32, kind="ExternalInput")
    d_af3 = nc.dram_tensor("af3", [128, nt, 3], F32, kind="ExternalInput")
    d_sfeat = nc.dram_tensor("sfeat", [s_total, CS], F32,
                             kind="ExternalInput")
    d_ofT = nc.dram_tensor("ofT", [CO, nq], BF16, kind="ExternalInput")
    d_w0T = nc.dram_tensor("w0T", [128, 3, C1], BF16, kind="ExternalInput")
    d_w1T = nc.dram_tensor("w1T", [128, 2, C2], BF16, kind="ExternalInput")
    d_g0 = nc.dram_tensor("g0p", [128, 2], F32, kind="ExternalInput")
    d_bt0 = nc.dram_tensor("bt0p", [128, 2], F32, kind="ExternalInput")
    d_g1 = nc.dram_tensor("g1p", [128, 2], F32, kind="ExternalInput")
    d_bt1 = nc.dram_tensor("bt1p", [128, 2], F32, kind="ExternalInput")
    d_eye = nc.dram_tensor("eye", [128, 128], F32, kind="ExternalInput")
    d_out = nc.dram_tensor("yT", [2, 128, nq], F32, kind="ExternalOutput")
    d_dbgi = nc.dram_tensor("dbgi", [128, nt, 3], F32, kind="ExternalOutput")
    d_dbgw = nc.dram_tensor("dbgw", [128, nt, 3], F32, kind="ExternalOutput")

    gs = 2
    with tile.TileContext(nc) as tc:
        with (
            tc.tile_pool(name="const", bufs=1) as cpool,
            tc.tile_pool(name="big", bufs=1) as bigp,
            tc.tile_pool(name="sc_sbE", bufs=2) as scpE,
            tc.tile_pool(name="sc_sbH", bufs=1) as scpH,
            tc.tile_pool(name="small", bufs=3) as smp,
            tc.tile_pool(name="gath", bufs=2) as gathp,
            tc.tile_pool(name="ps_sc", bufs=4, space="PSUM") as ps_sc,
            tc.tile_pool(name="ps_sm", bufs=2, space="PSUM") as ps_sm,
            tc.tile_pool(name="ps_mm1", bufs=2, space="PSUM") as ps_mm1,
            tc.tile_pool(name="dram", bufs=1, space="DRAM") as dramp,
        ):
            def load(pool, name, dram, shape, dt=F32):
                t_ = pool.tile(shape, dt, tag=name)
                nc.sync.dma_start(t_[:], dram[:])
                return t_

            eye = load(cpool, "eye", d_eye, [128, 128])
            w0T = load(cpool, "w0T", d_w0T, [128, 3, C1], BF16)
            w1T = load(cpool, "w1T", d_w1T, [128, 2, C2], BF16)
            g0p = load(cpool, "g0p", d_g0, [128, 2])
            bt0p = load(cpool, "bt0p", d_bt0, [128, 2])
            g1p = load(cpool, "g1p", d_g1, [128, 2])
            bt1p = load(cpool, "bt1p", d_bt1, [128, 2])
            xn23 = load(cpool, "xn23", d_xn23, [128, nt, 3])
            af3 = load(cpool, "af3", d_af3, [128, nt, 3])
            lhsT = load(cpool, "lhsT", d_lhsT, [KROWS, nt, 128], BF16)

            y0a = bigp.tile([128, nq], F32, tag="y0a")
            y0b = bigp.tile([128, nq], F32, tag="y0b")
            idx_all = bigp.tile([128, nt, 3], F32, tag="idx_all")
            w_all = bigp.tile([128, nt, 3], F32, tag="w_all")
            v8a = bigp.tile([128, nt, 8], F32, tag="v8a")
            i8a = bigp.tile([128, nt, 8], U32, tag="i8a")

            # ---------------- KNN selection (in blocks for overlap) -------
            # per-block weight math + wrapped-index build lets the dataflow
            # scheduler start block 0's gather/interp/MLP0 while later
            # blocks' KNN scans are still running
            i8f = bigp.tile([128, nt, 3], F32, tag="i8f")
            d3a = bigp.tile([128, nt, 3], F32, tag="d3a")
            r3a = bigp.tile([128, nt, 3], F32, tag="r3a")
            rsa = bigp.tile([128, nt], F32, tag="rsa")
            idx16 = bigp.tile([128, nt, 3], mybir.dt.int16, tag="idx16")
            ncols = nt * 24
            wrapped = bigp.tile([128, ncols], mybir.dt.int16, tag="wrapped")
            BLK = 16
            for t0 in range(0, nt, BLK):
                t1 = min(t0 + BLK, nt)
                for t in range(t0, t1):
                    if t < n_easy:
                        wt = w_easy
                        rt = scpE.tile([KROWS, w_easy], BF16, tag="rhsEt")
                        nc.sync.dma_start(rt[:], d_rhsE[:, t, :])
                        sc = ps_sc.tile([128, 512], F32, tag="ps_score")
                        nc.tensor.matmul(sc[:, 0:wt], lhsT[:, t, :],
                                         rt[:], start=True, stop=True)
                    else:
                        wt = s_total
                        j = t - n_easy
                        rt = scpH.tile([KROWS, s_total], BF16, tag="rhsHt")
                        nc.sync.dma_start(rt[:], d_rhsH[:, j, :])
                        sc = scpH.tile([128, s_total], F32, tag="scoresH")
                        for q8 in range(s_total // 512):
                            ps = ps_sc.tile([128, 512], F32, tag="ps_score")
                            col = q8 * 512
                            nc.tensor.matmul(ps[:], lhsT[:, t, :],
                                             rt[:, col:col + 512],
                                             start=True, stop=True)
                            nc.scalar.copy(sc[:, col:col + 512], ps[:])

                    nc.vector.max(v8a[:, t, :], sc[:, 0:wt])
                    nc.vector.max_index(i8a[:, t, :], v8a[:, t, :],
                                        sc[:, 0:wt])

                # batched weights + index remap for this block
                bs = slice(t0, t1)
                nc.vector.tensor_copy(i8f[:, bs, :], i8a[:, bs, 0:3])
                nc.vector.tensor_tensor(out=idx_all[:, bs, :],
                                        in0=i8f[:, bs, :],
                                        in1=af3[:, bs, :], op=ALU.add)
                nc.vector.tensor_tensor(out=d3a[:, bs, :], in0=xn23[:, bs, :],
                                        in1=v8a[:, bs, 0:3], op=ALU.subtract)
                nc.vector.tensor_scalar_add(d3a[:, bs, :], d3a[:, bs, :],
                                            W_EPS)
                nc.vector.reciprocal(r3a[:, bs, :], d3a[:, bs, :])
                nc.vector.tensor_reduce(out=rsa[:, bs], in_=r3a[:, bs, :],
                                        axis=AX.X, op=ALU.add)
                nc.vector.reciprocal(rsa[:, bs], rsa[:, bs])
                for kk in range(3):
                    nc.vector.tensor_tensor(out=w_all[:, bs, kk],
                                            in0=r3a[:, bs, kk],
                                            in1=rsa[:, bs], op=ALU.mult)
                # 16-partition-wrapped int16 indices for dma_gather: flat
                # order i = t*384 + k*128 + q -> partition q%16, col
                # t*24 + k*8 + q//16 -> 8 strided DMAs per block
                nc.vector.tensor_copy(idx16[:, bs, :], idx_all[:, bs, :])
                for u in range(8):
                    nc.gpsimd.dma_start(
                        wrapped[0:16, t0 * 24 + u:t1 * 24:8].rearrange(
                            "p (t k) -> p t k", k=3),
                        idx16[16 * u:16 * (u + 1), bs, :])
                for rep in range(1, 8):
                    nc.gpsimd.dma_start(
                        wrapped[16 * rep:16 * (rep + 1), t0 * 24:t1 * 24],
                        wrapped[0:16, t0 * 24:t1 * 24])

            nc.sync.dma_start(d_dbgi[:], idx_all[:])
            nc.sync.dma_start(d_dbgw[:], w_all[:])

            # ---------------- gather + interp + matmul0 ----------------
            n_g = (nt + gs - 1) // gs
            for g in range(n_g):
                t0 = g * gs
                gt = min(gs, nt - t0)
                gbuf = gathp.tile([128, gs * 3, CS], F32, tag="gath")
                ofTg = smp.tile([CO, gs * 128], BF16, tag="ofTg")
                nc.sync.dma_start(ofTg[:, 0:gt * 128],
                                  d_ofT[:, t0 * 128:(t0 + gt) * 128])
                nc.gpsimd.dma_gather(
                    out_ap=gbuf[:, 0:gt * 3, :],
                    in_ap=d_sfeat[:],
                    idxs_ap=wrapped[:, t0 * 24:(t0 + gt) * 24],
                    num_idxs=gt * 384,
                    num_idxs_reg=gt * 384,
                    elem_size=CS,
                )
                for tt in range(gt):
                    t = t0 + tt
                    interp = smp.tile([128, CS], F32, tag="interp")
                    acc = smp.tile([128, CS], F32, tag="interp_acc")
                    nc.vector.tensor_scalar(
                        out=acc[:], in0=gbuf[:, tt * 3, :],
                        scalar1=w_all[:, t, 0:1], scalar2=None, op0=ALU.mult)
                    nc.vector.scalar_tensor_tensor(
                        out=interp[:], in0=gbuf[:, tt * 3 + 1, :],
                        scalar=w_all[:, t, 1:2], in1=acc[:],
                        op0=ALU.mult, op1=ALU.add)
                    nc.vector.scalar_tensor_tensor(
                        out=interp[:], in0=gbuf[:, tt * 3 + 2, :],
                        scalar=w_all[:, t, 2:3], in1=interp[:],
                        op0=ALU.mult, op1=ALU.add)
                    iT = smp.tile([128, 2, 128], BF16, tag="interpT")
                    for hh in range(2):
                        ps_tr = ps_sm.tile([128, 128], F32, tag="ps_small")
                        nc.tensor.transpose(
                            ps_tr[:], interp[:, hh * 128:(hh + 1) * 128],
                            eye[:])
                        nc.vector.tensor_copy(iT[:, hh, :], ps_tr[:])
                    for m, ybuf in ((0, y0a), (1, y0b)):
                        ps_y = ps_sm.tile([128, 128], F32, tag="ps_small")
                        mcol = slice(m * 128, (m + 1) * 128)
                        nc.tensor.matmul(ps_y[:], w0T[:, 0, mcol],
                                         ofTg[:, tt * 128:(tt + 1) * 128],
                                         start=True, stop=False)
                        nc.tensor.matmul(ps_y[:], w0T[:, 1, mcol],
                                         iT[:, 0, :], start=False, stop=False)
                        nc.tensor.matmul(ps_y[:], w0T[:, 2, mcol],
                                         iT[:, 1, :], start=False, stop=True)
                        nc.scalar.copy(ybuf[:, t * 128:(t + 1) * 128], ps_y[:])

            # ---------------- BN helpers ----------------
            csz = divisor_csz(nq)

            def bn_allreduce(ya, yb, gp, btp, tag):
                """Returns (a, bhat) [128,2] with yhat = Relu(y*a + bhat)."""
                nchunk = nq // csz
                sump = smp.tile([128, 2 * nchunk], F32, tag=f"sump{tag}")
                sqp = smp.tile([128, 2 * nchunk], F32, tag=f"sqp{tag}")
                scratch = cpool.tile([128, csz], F32, tag="bn_scratch")
                for m, ybuf in ((0, ya), (1, yb)):
                    for ch in range(nchunk):
                        sl = slice(ch * csz, (ch + 1) * csz)
                        col = m * nchunk + ch
                        nc.vector.tensor_reduce(
                            out=sump[:, col:col + 1], in_=ybuf[:, sl],
                            axis=AX.X, op=ALU.add)
                        nc.scalar.activation(
                            out=scratch[:], in_=ybuf[:, sl], func=ACT.Square,
                            accum_out=sqp[:, col:col + 1])
                stats = smp.tile([128, 4], F32, tag=f"stats{tag}")
                nc.vector.tensor_reduce(
                    out=stats[:, 0:2],
                    in_=sump[:].rearrange("p (m c) -> p m c", m=2),
                    axis=AX.X, op=ALU.add)
                nc.vector.tensor_reduce(
                    out=stats[:, 2:4],
                    in_=sqp[:].rearrange("p (m c) -> p m c", m=2),
                    axis=AX.X, op=ALU.add)
                bi = dramp.tile([128, 4], F32, tag=f"bi{tag}")
                bo = dramp.tile([128, 4], F32, tag=f"bo{tag}")
                nc.gpsimd.dma_start(bi[:], stats[:])
                nc.gpsimd.collective_compute(
                    "AllReduce", ALU.add,
                    replica_groups=[list(range(n_cores))],
                    ins=[bi.opt()], outs=[bo.opt()])
                gstats = smp.tile([128, 4], F32, tag=f"gstats{tag}")
                nc.gpsimd.dma_start(gstats[:], bo[:])

                mean = smp.tile([128, 2], F32, tag=f"mean{tag}")
                nc.vector.tensor_scalar_mul(mean[:], gstats[:, 0:2],
                                            1.0 / n_points_total)
                vpe = smp.tile([128, 2], F32, tag=f"vpe{tag}")
                nc.vector.tensor_scalar_mul(vpe[:], gstats[:, 2:4],
                                            1.0 / n_points_total)
                msq = smp.tile([128, 2], F32, tag=f"msq{tag}")
                nc.vector.tensor_tensor(out=msq[:], in0=mean[:], in1=mean[:],
                                        op=ALU.mult)
                nc.vector.tensor_tensor(out=vpe[:], in0=vpe[:], in1=msq[:],
                                        op=ALU.subtract)
                nc.vector.tensor_scalar_add(vpe[:], vpe[:], BN_EPS)
                rcp = smp.tile([128, 2], F32, tag=f"rcp{tag}")
                nc.vector.reciprocal(rcp[:], vpe[:])
                rsq = smp.tile([128, 2], F32, tag=f"rsq{tag}")
                nc.scalar.activation(out=rsq[:], in_=rcp[:], func=ACT.Sqrt)
                t1 = smp.tile([128, 2], F32, tag=f"t1{tag}")
                nc.vector.tensor_tensor(out=t1[:], in0=rsq[:], in1=rsq[:],
                                        op=ALU.mult)
                nc.vector.tensor_tensor(out=t1[:], in0=t1[:], in1=vpe[:],
                                        op=ALU.mult)
                nc.vector.tensor_scalar(out=t1[:], in0=t1[:], scalar1=-0.5,
                                        scalar2=1.5, op0=ALU.mult, op1=ALU.add)
                nc.vector.tensor_tensor(out=rsq[:], in0=rsq[:], in1=t1[:],
                                        op=ALU.mult)
                a = smp.tile([128, 2], F32, tag=f"a{tag}")
                nc.vector.tensor_tensor(out=a[:], in0=gp[:], in1=rsq[:],
                                        op=ALU.mult)
                bhat = smp.tile([128, 2], F32, tag=f"bhat{tag}")
                nc.vector.tensor_tensor(out=bhat[:], in0=mean[:], in1=a[:],
                                        op=ALU.mult)
                nc.vector.tensor_tensor(out=bhat[:], in0=btp[:], in1=bhat[:],
                                        op=ALU.subtract)
                return a, bhat

            # ---------------- layer 1 ----------------
            a0, b0h = bn_allreduce(y0a, y0b, g0p, bt0p, "0")
            for ch in range(nq // csz):
                sl = slice(ch * csz, (ch + 1) * csz)
                yh0 = smp.tile([128, csz], BF16, tag="yh0")
                yh1 = smp.tile([128, csz], BF16, tag="yh1")
                nc.scalar.activation(out=yh0[:], in_=y0a[:, sl], func=ACT.Relu,
                                     scale=a0[:, 0:1], bias=b0h[:, 0:1])
                nc.scalar.activation(out=yh1[:], in_=y0b[:, sl], func=ACT.Relu,
                                     scale=a0[:, 1:2], bias=b0h[:, 1:2])
                for m, ybuf in ((0, y0a), (1, y0b)):
                    ps1 = ps_mm1.tile([128, csz], F32, tag="ps_mm1")
                    mcol = slice(m * 128, (m + 1) * 128)
                    nc.tensor.matmul(ps1[:], w1T[:, 0, mcol], yh0[:],
                                     start=True, stop=False)
                    nc.tensor.matmul(ps1[:], w1T[:, 1, mcol], yh1[:],
                                     start=False, stop=True)
                    nc.scalar.copy(ybuf[:, sl], ps1[:])

            # ---------------- layer 2 BN + out ----------------
            a1, b1h = bn_allreduce(y0a, y0b, g1p, bt1p, "1")
            for ch in range(nq // csz):
                sl = slice(ch * csz, (ch + 1) * csz)
                for m, ybuf in ((0, y0a), (1, y0b)):
                    o = smp.tile([128, csz], F32, tag="outsb")
                    nc.scalar.activation(out=o[:], in_=ybuf[:, sl],
                                         func=ACT.Relu,
                                         scale=a1[:, m:m + 1],
                                         bias=b1h[:, m:m + 1])
                    nc.sync.dma_start(d_out[m, :, sl], o[:])

    nc.compile()
    return nc


# ======================= host-side preparation =======================

def _hilbert_d3(x, y, z, order=10):
    X = np.stack([x, y, z], axis=0).astype(np.uint32).copy()
    M = np.uint32(1 << (order - 1))
    Q = M
    while Q > 1:
        P = np.uint32(Q - 1)
        for i in range(3):
            cond = (X[i] & Q) != 0
            X[0] = np.where(cond, X[0] ^ P, X[0])
            t = (X[0] ^ X[i]) & P
            X[0] ^= t
            X[i] ^= t
        Q >>= 1
    for i in range(1, 3):
        X[i] ^= X[i - 1]
    t2 = np.zeros_like(X[0])
    Q = M
    while Q > 1:
        t2 = np.where((X[2] & Q) != 0, t2 ^ np.uint32(Q - 1), t2)
        Q >>= 1
    for i in range(3):
        X[i] ^= t2
    key = np.zeros(X.shape[1], dtype=np.uint64)
    for b in range(order - 1, -1, -1):
        for i in range(3):
            key = ((key << np.uint64(1))
                   | ((X[i] >> np.uint32(b)) & np.uint32(1)).astype(np.uint64))
    return key


def _hkeys(pts, lo, hi, order=10):
    qq = ((pts - lo) / np.maximum(hi - lo, 1e-9)).clip(0.0, 1.0)
    qq = (qq * ((1 << order) - 1)).astype(np.uint32)
    return _hilbert_d3(qq[:, 0], qq[:, 1], qq[:, 2], order)


def _bf16(x):
    return np.asarray(np.asarray(x, np.float32), ml_dtypes.bfloat16)


def _dec3(x):
    """fp64 -> three bf16 terms (residual ~2^-24 |x|)."""
    h = _bf16(x)
    r = x - np.asarray(h, np.float64)
    m = _bf16(r)
    r2 = r - np.asarray(m, np.float64)
    return h, m, _bf16(r2)


def _score_rows(u, v):
    """u: [nq, 3] fp64 query offsets; v: [w, 3] fp64 sample offsets.
    Returns L [27, nq] bf16, R [27, w] bf16 with sum_k L[k] outer R[k]
    ~= 2 u.v - |v|^2 (error ~1e-7 for |u|,|v| <~ 1)."""
    uh, um, ul = _dec3(2.0 * u)
    vh, vm, vl = _dec3(v)
    n = v * v
    n1, n2, n3 = _dec3(-n)
    one = np.ones(u.shape[0], ml_dtypes.bfloat16)
    L, R = [], []
    for ci in range(3):       # hi block: products ~O(r^2), cancel early
        L += [uh[:, ci], one]
        R += [vh[:, ci], n1[:, ci]]
    for ci in range(3):       # med block
        L += [um[:, ci], uh[:, ci], one]
        R += [vh[:, ci], vm[:, ci], n2[:, ci]]
    for ci in range(3):       # lo block
        L += [ul[:, ci], uh[:, ci], um[:, ci], one]
        R += [vh[:, ci], vl[:, ci], vm[:, ci], n3[:, ci]]
    return (np.ascontiguousarray(np.stack(L)),
            np.ascontiguousarray(np.stack(R)))


def _pack_core(q, ss, glo, ghi, w_easy, n_easy, n_hard, s_total):
    """q: [nq_real, 3] fp64 queries (this core); ss: [s_total, 3] fp64
    samples sorted by Hilbert key on the (glo, ghi) grid.  Returns (perm,
    a_t) where perm is the slot -> local-query-index map (len nt*128, with
    duplicate pads) and a_t the per-easy-tile window starts."""
    nq_real = q.shape[0]
    nt = n_easy + n_hard
    slots = nt * 128
    e_slots, h_slots = n_easy * 128, n_hard * 128

    qk = _hkeys(q.astype(np.float32), glo, ghi)
    sk = _hkeys(ss.astype(np.float32), glo, ghi)
    oq = np.argsort(qk, kind="stable")
    # ss must already be sorted by its key for searchsorted windows
    qs = q[oq]
    pos = np.searchsorted(np.sort(sk), qk[oq])
    ncand = min(48, s_total)
    start = (pos - ncand // 2).clip(0, s_total - ncand)
    cand = start[:, None] + np.arange(ncand)[None, :]
    dc = ((qs[:, None, :] - ss[cand]) ** 2).sum(-1)
    r3sq = np.sort(dc, 1)[:, 2] * (1 + 1e-5) + 2e-6
    # window per query from full distance matrix (fp32 blas, with margin)
    qf, sf = qs.astype(np.float32), ss.astype(np.float32)
    D = ((qf ** 2).sum(-1)[:, None] + (sf ** 2).sum(-1)[None, :]
         - 2.0 * (qf @ sf.T)).astype(np.float64)
    within = D <= (r3sq[:, None] + 4e-6)
    first = np.argmax(within, 1)
    last = s_total - 1 - np.argmax(within[:, ::-1], 1)
    wq = last - first + 1

    easy = [i for i in range(nq_real) if wq[i] <= w_easy]
    hard = [i for i in range(nq_real) if wq[i] > w_easy]

    def greedy(lst):
        """Pack hilbert-ordered queries into tiles with union <= w_easy."""
        tiles, cur, lo, hi = [], [], None, None
        for i in lst:
            nlo = first[i] if lo is None else min(lo, first[i])
            nhi = last[i] if hi is None else max(hi, last[i])
            if len(cur) < 128 and nhi - nlo + 1 <= w_easy:
                cur.append(i)
                lo, hi = nlo, nhi
            else:
                tiles.append(cur)
                cur, lo, hi = [i], first[i], last[i]
        if cur:
            tiles.append(cur)
        return tiles

    for _ in range(400):
        if len(easy) > e_slots:
            drop = set(sorted(easy, key=lambda i: wq[i])[e_slots:])
            hard += [i for i in easy if i in drop]
            easy = [i for i in easy if i not in drop]
        tiles = greedy(easy)
        if len(tiles) <= n_easy:
            break
        drop = set(sorted(easy, key=lambda i: wq[i])[-8:])
        hard += [i for i in easy if i in drop]
        easy = [i for i in easy if i not in drop]
    if len(hard) > h_slots:
        # degraded fallback (windows of forced-back queries get clamped);
        # should not trigger on sane inputs
        back = set(sorted(hard, key=lambda i: wq[i])[:len(hard) - h_slots])
        easy = sorted(easy + [i for i in hard if i in back],
                      key=lambda i: int(np.searchsorted(np.sort(qk), qk[oq[i]])))
        hard = [i for i in hard if i not in back]
        tiles = greedy(easy)[:n_easy]
        placed = set()
        for tl in tiles:
            placed |= set(tl)
        hard += [i for i in easy if i not in placed]
        easy = [i for i in easy if i in placed]
        hard = hard[:h_slots]

    def pad128(tl):
        # cyclic duplication spreads pad weight evenly over the tile's
        # members (pads are counted in the BN-stat population)
        base = list(tl)
        j = 0
        while len(tl) < 128:
            tl.append(base[j % len(base)])
            j += 1
        return tl

    tiles = [pad128(tl) for tl in tiles]
    while len(tiles) < n_easy:
        tiles.append(list(tiles[-1]))
    a_t = []
    for tl in tiles:
        lo = min(first[i] for i in tl)
        a_t.append(int(min(lo, s_total - w_easy)))

    hard = sorted(hard, key=lambda i: wq[i])
    while len(hard) < h_slots:
        hard.append((len(hard) * 97 + 13) % nq_real)

    flat_easy = [i for tl in tiles for i in tl]
    perm = np.array([oq[i] for i in flat_easy] + [oq[i] for i in hard])
    assert perm.shape[0] == slots
    return perm, a_t


_PROGRAM_CACHE = {}


def make_core_inputs(sampled_xyz, sampled_features, original_xyz,
                     original_features, w0, w1, g0, bt0, g1, bt1, core,
                     n_easy=N_EASY, n_hard=N_HARD, w_easy=W_EASY):
    b, h = core // 2, core % 2
    s_total = sampled_xyz.shape[1]
    nq_real = original_xyz.shape[1] // 2
    nt = n_easy + n_hard
    nq = nt * 128
    f32 = np.float32

    s64 = np.asarray(sampled_xyz[b], np.float64)
    allp = np.vstack([np.asarray(original_xyz[b], f32),
                      np.asarray(sampled_xyz[b], f32)])
    glo, ghi = allp.min(0), allp.max(0)
    os_ = np.argsort(_hkeys(s64.astype(f32), glo, ghi), kind="stable")
    ss = s64[os_]
    sfeat = np.ascontiguousarray(
        np.asarray(sampled_features[b], f32)[os_])

    q = np.asarray(original_xyz[b, h * nq_real:(h + 1) * nq_real], np.float64)
    of = np.asarray(original_features[b, h * nq_real:(h + 1) * nq_real], f32)

    perm, a_t = _pack_core(q, ss, glo, ghi, w_easy, n_easy, n_hard, s_total)
    a_t = list(a_t) + [0] * n_hard

    lhsT = np.zeros((KROWS, nt, 128), ml_dtypes.bfloat16)
    rhsE = np.zeros((KROWS, max(n_easy, 1), w_easy), ml_dtypes.bfloat16)
    rhsH = np.zeros((KROWS, max(n_hard, 1), s_total), ml_dtypes.bfloat16)
    xn2p = np.zeros((128, nt), f32)  # expanded to xn23 below
    for t in range(nt):
        ql = perm[t * 128:(t + 1) * 128]
        qt = q[ql]
        c = qt.mean(0)
        u = qt - c
        if t < n_easy:
            v = ss[a_t[t]:a_t[t] + w_easy] - c
            L, R = _score_rows(u, v)
            rhsE[:, t, :] = R
        else:
            v = ss - c
            L, R = _score_rows(u, v)
            rhsH[:, t - n_easy, :] = R
        lhsT[:, t, :] = L
        xn2p[:, t] = (u * u).sum(-1).astype(f32)

    of_perm = of[perm]                                   # [nq, CO]
    af = np.repeat(np.asarray(a_t, f32)[None, :], 128, 0)
    return {
        "lhsT": lhsT,
        "rhsE": rhsE,
        "rhsH": rhsH,
        "xn23": np.ascontiguousarray(
            np.repeat(xn2p[:, :, None], 3, axis=2)),
        "af3": np.ascontiguousarray(
            np.repeat(af[:, :, None], 3, axis=2)),
        "sfeat": sfeat,
        "ofT": _bf16(np.ascontiguousarray(of_perm.T)),
        "w0T": _bf16(np.ascontiguousarray(
            w0.T.reshape(3, 128, C1).transpose(1, 0, 2))),
        "w1T": _bf16(np.ascontiguousarray(
            w1.T.reshape(2, 128, C2).transpose(1, 0, 2))),
        "g0p": np.ascontiguousarray(g0.reshape(2, 128).T).astype(f32),
        "bt0p": np.ascontiguousarray(bt0.reshape(2, 128).T).astype(f32),
        "g1p": np.ascontiguousarray(g1.reshape(2, 128).T).astype(f32),
        "bt1p": np.ascontiguousarray(bt1.reshape(2, 128).T).astype(f32),
        "eye": np.eye(128, dtype=f32),
    }, perm


def kernel(sampled_xyz, sampled_features, original_xyz, original_features,
           w0, b0, g0, bt0, w1, b1, g1, bt1, k):
    assert int(k) == 3
    from concourse.bass_utils import run_bass_kernel_spmd

    key = "full"
    if key not in _PROGRAM_CACHE:
        _PROGRAM_CACHE[key] = build_program()
    nc = _PROGRAM_CACHE[key]

    args = (sampled_xyz, sampled_features, original_xyz, original_features,
            w0, w1, g0, bt0, g1, bt1)
    args = [np.asarray(a, np.float32) for a in args]
    in_maps, perms = [], []
    for c in range(NCORES):
        im, perm = make_core_inputs(*args, core=c)
        in_maps.append(im)
        perms.append(perm)
    res = run_bass_kernel_spmd(nc, in_maps, core_ids=list(range(NCORES)))
    out = np.empty((B, N, C2), np.float32)
    nq = NT * 128
    for c in range(NCORES):
        b, h = c // 2, c % 2
        yT = res.results[c]["yT"]            # [2, 128, nq]
        y = yT.reshape(256, nq).T            # [nq, 256]
        inv = np.zeros(QP, np.int64)
        inv[perms[c][::-1]] = np.arange(nq)[::-1]  # first occurrence wins
        out[b, h * QP:(h + 1) * QP] = y[inv]
    return out


# revision 35
# speedup vs baseline: 1.1507x; 1.0695x over previous
"""PointNet++ FeaturePropagation Trainium2 kernel (8-core SPMD).

Per core c of 8: batch b = c//2, query-half h = c%2 (8192 original points).
KNN scores are computed on the PE as a 27-row bf16-triple decomposition of
2*(q-c_t).(s-c_t) - |s-c_t|^2 per query tile (center c_t), which makes every
product exact and keeps fp32 PSUM accumulation error ~1e-7 -- required
because 3rd/4th-neighbor distance gaps go down to ~1e-7 on this data.
Queries are Hilbert-sorted on the host; each 128-query tile scores against
a contiguous window of W=512 Hilbert-sorted samples (provably containing
the true 3-NN via a cheap host-side 3rd-NN upper bound); outlier queries
that need wider windows go to 3 dense tiles scored against all 4096
samples.  DVE max/max_index extract top-8; inverse-distance weights and
the gather/interp/MLP/BatchNorm pipeline follow (BN stats via 8-core
AllReduce; conv biases cancel through BN and are skipped).
Host does layout transforms (sorting/transpose/decomposition) and the
final unshard/unpermute.
"""

import numpy as np
import ml_dtypes

import concourse.bass as bass
import concourse.bacc as bacc
import concourse.mybir as mybir
import concourse.tile as tile

F32 = mybir.dt.float32
BF16 = mybir.dt.bfloat16
U32 = mybir.dt.uint32
ALU = mybir.AluOpType
ACT = mybir.ActivationFunctionType
AX = mybir.AxisListType

B, S, N = 4, 4096, 16384
CS, CO = 256, 128
C1, C2 = 256, 256
NCORES = 8
QP = N // 2          # real queries per core
N_EASY, N_HARD, W_EASY = 62, 3, 512
NT = N_EASY + N_HARD  # 65 tiles -> 8320 slots (128 duplicate pads)
BN_EPS = 1e-5
W_EPS = 1e-8
KROWS = 27


def divisor_csz(nq):
    for d in range(512, 0, -1):
        if nq % d == 0:
            return d


def build_program(n_easy=N_EASY, n_hard=N_HARD, w_easy=W_EASY, s_total=S,
                  n_cores=NCORES, n_points_total=None):
    nt = n_easy + n_hard
    nq = nt * 128
    if n_points_total is None:
        n_points_total = n_cores * nq
    nc = bacc.Bacc("TRN2", target_bir_lowering=False, debug=False,
                   num_devices=n_cores)

    d_lhsT = nc.dram_tensor("lhsT", [KROWS, nt, 128], BF16,
                            kind="ExternalInput")
    d_rhsE = nc.dram_tensor("rhsE", [KROWS, max(n_easy, 1), w_easy], BF16,
                            kind="ExternalInput")
    d_rhsH = nc.dram_tensor("rhsH", [KROWS, max(n_hard, 1), s_total], BF16,
                            kind="ExternalInput")
    d_xn23 = nc.dram_tensor("xn23", [128, nt, 3], F32, kind="ExternalInput")
    d_af3 = nc.dram_tensor("af3", [128, nt, 3], F32, kind="ExternalInput")
    d_sfeat = nc.dram_tensor("sfeat", [s_total, CS], F32,
                             kind="ExternalInput")
    d_ofT = nc.dram_tensor("ofT", [CO, nq], BF16, kind="ExternalInput")
    d_w0T = nc.dram_tensor("w0T", [128, 3, C1], BF16, kind="ExternalInput")
    d_w1T = nc.dram_tensor("w1T", [128, 2, C2], BF16, kind="ExternalInput")
    d_g0 = nc.dram_tensor("g0p", [128, 2], F32, kind="ExternalInput")
    d_bt0 = nc.dram_tensor("bt0p", [128, 2], F32, kind="ExternalInput")
    d_g1 = nc.dram_tensor("g1p", [128, 2], F32, kind="ExternalInput")
    d_bt1 = nc.dram_tensor("bt1p", [128, 2], F32, kind="ExternalInput")
    d_eye = nc.dram_tensor("eye", [128, 128], F32, kind="ExternalInput")
    d_out = nc.dram_tensor("yT", [2, 128, nq], F32, kind="ExternalOutput")
    d_dbgi = nc.dram_tensor("dbgi", [128, nt, 3], F32, kind="ExternalOutput")
    d_dbgw = nc.dram_tensor("dbgw", [128, nt, 3], F32, kind="ExternalOutput")

    gs = 2
    with tile.TileContext(nc) as tc:
        with (
            tc.tile_pool(name="const", bufs=1) as cpool,
            tc.tile_pool(name="big", bufs=1) as bigp,
            tc.tile_pool(name="sc_sbE", bufs=2) as scpE,
            tc.tile_pool(name="sc_sbH", bufs=1) as scpH,
            tc.tile_pool(name="small", bufs=3) as smp,
            tc.tile_pool(name="gath", bufs=2) as gathp,
            tc.tile_pool(name="ps_sc", bufs=4, space="PSUM") as ps_sc,
            tc.tile_pool(name="ps_sm", bufs=2, space="PSUM") as ps_sm,
            tc.tile_pool(name="ps_mm1", bufs=2, space="PSUM") as ps_mm1,
            tc.tile_pool(name="dram", bufs=1, space="DRAM") as dramp,
        ):
            def load(pool, name, dram, shape, dt=F32):
                t_ = pool.tile(shape, dt, tag=name)
                nc.sync.dma_start(t_[:], dram[:])
                return t_

            eye = load(cpool, "eye", d_eye, [128, 128])
            w0T = load(cpool, "w0T", d_w0T, [128, 3, C1], BF16)
            w1T = load(cpool, "w1T", d_w1T, [128, 2, C2], BF16)
            g0p = load(cpool, "g0p", d_g0, [128, 2])
            bt0p = load(cpool, "bt0p", d_bt0, [128, 2])
            g1p = load(cpool, "g1p", d_g1, [128, 2])
            bt1p = load(cpool, "bt1p", d_bt1, [128, 2])
            xn23 = load(cpool, "xn23", d_xn23, [128, nt, 3])
            af3 = load(cpool, "af3", d_af3, [128, nt, 3])
            lhsT = load(cpool, "lhsT", d_lhsT, [KROWS, nt, 128], BF16)

            y0a = bigp.tile([128, nq], F32, tag="y0a")
            y0b = bigp.tile([128, nq], F32, tag="y0b")
            idx_all = bigp.tile([128, nt, 3], F32, tag="idx_all")
            w_all = bigp.tile([128, nt, 3], F32, tag="w_all")
            v8a = bigp.tile([128, nt, 8], F32, tag="v8a")
            i8a = bigp.tile([128, nt, 8], U32, tag="i8a")

            # ---------------- KNN selection (in blocks for overlap) -------
            # per-block weight math + wrapped-index build lets the dataflow
            # scheduler start block 0's gather/interp/MLP0 while later
            # blocks' KNN scans are still running
            i8f = bigp.tile([128, nt, 3], F32, tag="i8f")
            d3a = bigp.tile([128, nt, 3], F32, tag="d3a")
            r3a = bigp.tile([128, nt, 3], F32, tag="r3a")
            rsa = bigp.tile([128, nt], F32, tag="rsa")
            idx16 = bigp.tile([128, nt, 3], mybir.dt.int16, tag="idx16")
            ncols = nt * 24
            wrapped = bigp.tile([128, ncols], mybir.dt.int16, tag="wrapped")
            BLK = 16
            for t0 in range(0, nt, BLK):
                t1 = min(t0 + BLK, nt)
                for t in range(t0, t1):
                    if t < n_easy:
                        wt = w_easy
                        rt = scpE.tile([KROWS, w_easy], BF16, tag="rhsEt")
                        nc.sync.dma_start(rt[:], d_rhsE[:, t, :])
                        sc = ps_sc.tile([128, 512], F32, tag="ps_score")
                        nc.tensor.matmul(sc[:, 0:wt], lhsT[:, t, :],
                                         rt[:], start=True, stop=True)
                    else:
                        wt = s_total
                        j = t - n_easy
                        rt = scpH.tile([KROWS, s_total], BF16, tag="rhsHt")
                        nc.sync.dma_start(rt[:], d_rhsH[:, j, :])
                        sc = scpH.tile([128, s_total], F32, tag="scoresH")
                        for q8 in range(s_total // 512):
                            ps = ps_sc.tile([128, 512], F32, tag="ps_score")
                            col = q8 * 512
                            nc.tensor.matmul(ps[:], lhsT[:, t, :],
                                             rt[:, col:col + 512],
                                             start=True, stop=True)
                            nc.scalar.copy(sc[:, col:col + 512], ps[:])

                    nc.vector.max(v8a[:, t, :], sc[:, 0:wt])
                    nc.vector.max_index(i8a[:, t, :], v8a[:, t, :],
                                        sc[:, 0:wt])

                # batched weights + index remap for this block
                bs = slice(t0, t1)
                nc.vector.tensor_copy(i8f[:, bs, :], i8a[:, bs, 0:3])
                nc.vector.tensor_tensor(out=idx_all[:, bs, :],
                                        in0=i8f[:, bs, :],
                                        in1=af3[:, bs, :], op=ALU.add)
                nc.vector.tensor_tensor(out=d3a[:, bs, :], in0=xn23[:, bs, :],
                                        in1=v8a[:, bs, 0:3], op=ALU.subtract)
                nc.vector.tensor_scalar_add(d3a[:, bs, :], d3a[:, bs, :],
                                            W_EPS)
                nc.vector.reciprocal(r3a[:, bs, :], d3a[:, bs, :])
                nc.vector.tensor_reduce(out=rsa[:, bs], in_=r3a[:, bs, :],
                                        axis=AX.X, op=ALU.add)
                nc.vector.reciprocal(rsa[:, bs], rsa[:, bs])
                for kk in range(3):
                    nc.vector.tensor_tensor(out=w_all[:, bs, kk],
                                            in0=r3a[:, bs, kk],
                                            in1=rsa[:, bs], op=ALU.mult)
                # 16-partition-wrapped int16 indices for dma_gather: flat
                # order i = t*384 + k*128 + q -> partition q%16, col
                # t*24 + k*8 + q//16 -> 8 strided DMAs per block
                nc.vector.tensor_copy(idx16[:, bs, :], idx_all[:, bs, :])
                for u in range(8):
                    nc.gpsimd.dma_start(
                        wrapped[0:16, t0 * 24 + u:t1 * 24:8].rearrange(
                            "p (t k) -> p t k", k=3),
                        idx16[16 * u:16 * (u + 1), bs, :])
                for rep in range(1, 8):
                    nc.gpsimd.dma_start(
                        wrapped[16 * rep:16 * (rep + 1), t0 * 24:t1 * 24],
                        wrapped[0:16, t0 * 24:t1 * 24])

            nc.sync.dma_start(d_dbgi[:], idx_all[:])
            nc.sync.dma_start(d_dbgw[:], w_all[:])

            # ---------------- gather + interp + matmul0 ----------------
            n_g = (nt + gs - 1) // gs
            for g in range(n_g):
                t0 = g * gs
                gt = min(gs, nt - t0)
                gbuf = gathp.tile([128, gs * 3, CS], F32, tag="gath")
                ofTg = smp.tile([CO, gs * 128], BF16, tag="ofTg")
                nc.sync.dma_start(ofTg[:, 0:gt * 128],
                                  d_ofT[:, t0 * 128:(t0 + gt) * 128])
                nc.gpsimd.dma_gather(
                    out_ap=gbuf[:, 0:gt * 3, :],
                    in_ap=d_sfeat[:],
                    idxs_ap=wrapped[:, t0 * 24:(t0 + gt) * 24],
                    num_idxs=gt * 384,
                    num_idxs_reg=gt * 384,
                    elem_size=CS,
                )
                for tt in range(gt):
                    t = t0 + tt
                    interp = smp.tile([128, CS], F32, tag="interp")
                    acc = smp.tile([128, CS], F32, tag="interp_acc")
                    nc.vector.tensor_scalar(
                        out=acc[:], in0=gbuf[:, tt * 3, :],
                        scalar1=w_all[:, t, 0:1], scalar2=None, op0=ALU.mult)
                    nc.vector.scalar_tensor_tensor(
                        out=interp[:], in0=gbuf[:, tt * 3 + 1, :],
                        scalar=w_all[:, t, 1:2], in1=acc[:],
                        op0=ALU.mult, op1=ALU.add)
                    nc.vector.scalar_tensor_tensor(
                        out=interp[:], in0=gbuf[:, tt * 3 + 2, :],
                        scalar=w_all[:, t, 2:3], in1=interp[:],
                        op0=ALU.mult, op1=ALU.add)
                    iT = smp.tile([128, 2, 128], BF16, tag="interpT")
                    for hh in range(2):
                        ps_tr = ps_sm.tile([128, 128], F32, tag="ps_small")
                        nc.tensor.transpose(
                            ps_tr[:], interp[:, hh * 128:(hh + 1) * 128],
                            eye[:])
                        nc.scalar.copy(iT[:, hh, :], ps_tr[:])
                    for m, ybuf in ((0, y0a), (1, y0b)):
                        ps_y = ps_sm.tile([128, 128], F32, tag="ps_small")
                        mcol = slice(m * 128, (m + 1) * 128)
                        nc.tensor.matmul(ps_y[:], w0T[:, 0, mcol],
                                         ofTg[:, tt * 128:(tt + 1) * 128],
                                         start=True, stop=False)
                        nc.tensor.matmul(ps_y[:], w0T[:, 1, mcol],
                                         iT[:, 0, :], start=False, stop=False)
                        nc.tensor.matmul(ps_y[:], w0T[:, 2, mcol],
                                         iT[:, 1, :], start=False, stop=True)
                        nc.scalar.copy(ybuf[:, t * 128:(t + 1) * 128], ps_y[:])

            # ---------------- BN helpers ----------------
            csz = divisor_csz(nq)

            def bn_allreduce(ya, yb, gp, btp, tag):
                """Returns (a, bhat) [128,2] with yhat = Relu(y*a + bhat)."""
                nchunk = 8
                csz_s = nq // nchunk
                sump = smp.tile([128, 2 * nchunk], F32, tag=f"sump{tag}")
                sqp = smp.tile([128, 2 * nchunk], F32, tag=f"sqp{tag}")
                scratch = cpool.tile([128, csz_s], F32, tag="bn_scratch")
                for m, ybuf in ((0, ya), (1, yb)):
                    for ch in range(nchunk):
                        sl = slice(ch * csz_s, (ch + 1) * csz_s)
                        col = m * nchunk + ch
                        nc.vector.tensor_reduce(
                            out=sump[:, col:col + 1], in_=ybuf[:, sl],
                            axis=AX.X, op=ALU.add)
                        nc.scalar.activation(
                            out=scratch[:], in_=ybuf[:, sl], func=ACT.Square,
                            accum_out=sqp[:, col:col + 1])
                stats = smp.tile([128, 4], F32, tag=f"stats{tag}")
                nc.vector.tensor_reduce(
                    out=stats[:, 0:2],
                    in_=sump[:].rearrange("p (m c) -> p m c", m=2),
                    axis=AX.X, op=ALU.add)
                nc.vector.tensor_reduce(
                    out=stats[:, 2:4],
                    in_=sqp[:].rearrange("p (m c) -> p m c", m=2),
                    axis=AX.X, op=ALU.add)
                bi = dramp.tile([128, 4], F32, tag=f"bi{tag}")
                bo = dramp.tile([128, 4], F32, tag=f"bo{tag}")
                nc.gpsimd.dma_start(bi[:], stats[:])
                nc.gpsimd.collective_compute(
                    "AllReduce", ALU.add,
                    replica_groups=[list(range(n_cores))],
                    ins=[bi.opt()], outs=[bo.opt()])
                gstats = smp.tile([128, 4], F32, tag=f"gstats{tag}")
                nc.gpsimd.dma_start(gstats[:], bo[:])

                mean = smp.tile([128, 2], F32, tag=f"mean{tag}")
                nc.vector.tensor_scalar_mul(mean[:], gstats[:, 0:2],
                                            1.0 / n_points_total)
                vpe = smp.tile([128, 2], F32, tag=f"vpe{tag}")
                nc.vector.tensor_scalar_mul(vpe[:], gstats[:, 2:4],
                                            1.0 / n_points_total)
                msq = smp.tile([128, 2], F32, tag=f"msq{tag}")
                nc.vector.tensor_tensor(out=msq[:], in0=mean[:], in1=mean[:],
                                        op=ALU.mult)
                nc.vector.tensor_tensor(out=vpe[:], in0=vpe[:], in1=msq[:],
                                        op=ALU.subtract)
                nc.vector.tensor_scalar_add(vpe[:], vpe[:], BN_EPS)
                rcp = smp.tile([128, 2], F32, tag=f"rcp{tag}")
                nc.vector.reciprocal(rcp[:], vpe[:])
                rsq = smp.tile([128, 2], F32, tag=f"rsq{tag}")
                nc.scalar.activation(out=rsq[:], in_=rcp[:], func=ACT.Sqrt)
                t1 = smp.tile([128, 2], F32, tag=f"t1{tag}")
                nc.vector.tensor_tensor(out=t1[:], in0=rsq[:], in1=rsq[:],
                                        op=ALU.mult)
                nc.vector.tensor_tensor(out=t1[:], in0=t1[:], in1=vpe[:],
                                        op=ALU.mult)
                nc.vector.tensor_scalar(out=t1[:], in0=t1[:], scalar1=-0.5,
                                        scalar2=1.5, op0=ALU.mult, op1=ALU.add)
                nc.vector.tensor_tensor(out=rsq[:], in0=rsq[:], in1=t1[:],
                                        op=ALU.mult)
                a = smp.tile([128, 2], F32, tag=f"a{tag}")
                nc.vector.tensor_tensor(out=a[:], in0=gp[:], in1=rsq[:],
                                        op=ALU.mult)
                bhat = smp.tile([128, 2], F32, tag=f"bhat{tag}")
                nc.vector.tensor_tensor(out=bhat[:], in0=mean[:], in1=a[:],
                                        op=ALU.mult)
                nc.vector.tensor_tensor(out=bhat[:], in0=btp[:], in1=bhat[:],
                                        op=ALU.subtract)
                return a, bhat

            # ---------------- layer 1 ----------------
            a0, b0h = bn_allreduce(y0a, y0b, g0p, bt0p, "0")
            for ch in range(nq // csz):
                sl = slice(ch * csz, (ch + 1) * csz)
                yh0 = smp.tile([128, csz], BF16, tag="yh0")
                yh1 = smp.tile([128, csz], BF16, tag="yh1")
                nc.scalar.activation(out=yh0[:], in_=y0a[:, sl], func=ACT.Relu,
                                     scale=a0[:, 0:1], bias=b0h[:, 0:1])
                nc.scalar.activation(out=yh1[:], in_=y0b[:, sl], func=ACT.Relu,
                                     scale=a0[:, 1:2], bias=b0h[:, 1:2])
                for m, ybuf in ((0, y0a), (1, y0b)):
                    ps1 = ps_mm1.tile([128, csz], F32, tag="ps_mm1")
                    mcol = slice(m * 128, (m + 1) * 128)
                    nc.tensor.matmul(ps1[:], w1T[:, 0, mcol], yh0[:],
                                     start=True, stop=False)
                    nc.tensor.matmul(ps1[:], w1T[:, 1, mcol], yh1[:],
                                     start=False, stop=True)
                    nc.scalar.copy(ybuf[:, sl], ps1[:])

            # ---------------- layer 2 BN + out ----------------
            a1, b1h = bn_allreduce(y0a, y0b, g1p, bt1p, "1")
            for ch in range(nq // csz):
                sl = slice(ch * csz, (ch + 1) * csz)
                for m, ybuf in ((0, y0a), (1, y0b)):
                    o = smp.tile([128, csz], F32, tag="outsb")
                    nc.scalar.activation(out=o[:], in_=ybuf[:, sl],
                                         func=ACT.Relu,
                                         scale=a1[:, m:m + 1],
                                         bias=b1h[:, m:m + 1])
                    nc.sync.dma_start(d_out[m, :, sl], o[:])

    nc.compile()
    return nc


# ======================= host-side preparation =======================

def _hilbert_d3(x, y, z, order=10):
    X = np.stack([x, y, z], axis=0).astype(np.uint32).copy()
    M = np.uint32(1 << (order - 1))
    Q = M
    while Q > 1:
        P = np.uint32(Q - 1)
        for i in range(3):
            cond = (X[i] & Q) != 0
            X[0] = np.where(cond, X[0] ^ P, X[0])
            t = (X[0] ^ X[i]) & P
            X[0] ^= t
            X[i] ^= t
        Q >>= 1
    for i in range(1, 3):
        X[i] ^= X[i - 1]
    t2 = np.zeros_like(X[0])
    Q = M
    while Q > 1:
        t2 = np.where((X[2] & Q) != 0, t2 ^ np.uint32(Q - 1), t2)
        Q >>= 1
    for i in range(3):
        X[i] ^= t2
    key = np.zeros(X.shape[1], dtype=np.uint64)
    for b in range(order - 1, -1, -1):
        for i in range(3):
            key = ((key << np.uint64(1))
                   | ((X[i] >> np.uint32(b)) & np.uint32(1)).astype(np.uint64))
    return key


def _hkeys(pts, lo, hi, order=10):
    qq = ((pts - lo) / np.maximum(hi - lo, 1e-9)).clip(0.0, 1.0)
    qq = (qq * ((1 << order) - 1)).astype(np.uint32)
    return _hilbert_d3(qq[:, 0], qq[:, 1], qq[:, 2], order)


def _bf16(x):
    return np.asarray(np.asarray(x, np.float32), ml_dtypes.bfloat16)


def _dec3(x):
    """fp64 -> three bf16 terms (residual ~2^-24 |x|)."""
    h = _bf16(x)
    r = x - np.asarray(h, np.float64)
    m = _bf16(r)
    r2 = r - np.asarray(m, np.float64)
    return h, m, _bf16(r2)


def _score_rows(u, v):
    """u: [nq, 3] fp64 query offsets; v: [w, 3] fp64 sample offsets.
    Returns L [27, nq] bf16, R [27, w] bf16 with sum_k L[k] outer R[k]
    ~= 2 u.v - |v|^2 (error ~1e-7 for |u|,|v| <~ 1)."""
    uh, um, ul = _dec3(2.0 * u)
    vh, vm, vl = _dec3(v)
    n = v * v
    n1, n2, n3 = _dec3(-n)
    one = np.ones(u.shape[0], ml_dtypes.bfloat16)
    L, R = [], []
    for ci in range(3):       # hi block: products ~O(r^2), cancel early
        L += [uh[:, ci], one]
        R += [vh[:, ci], n1[:, ci]]
    for ci in range(3):       # med block
        L += [um[:, ci], uh[:, ci], one]
        R += [vh[:, ci], vm[:, ci], n2[:, ci]]
    for ci in range(3):       # lo block
        L += [ul[:, ci], uh[:, ci], um[:, ci], one]
        R += [vh[:, ci], vl[:, ci], vm[:, ci], n3[:, ci]]
    return (np.ascontiguousarray(np.stack(L)),
            np.ascontiguousarray(np.stack(R)))


def _pack_core(q, ss, glo, ghi, w_easy, n_easy, n_hard, s_total):
    """q: [nq_real, 3] fp64 queries (this core); ss: [s_total, 3] fp64
    samples sorted by Hilbert key on the (glo, ghi) grid.  Returns (perm,
    a_t) where perm is the slot -> local-query-index map (len nt*128, with
    duplicate pads) and a_t the per-easy-tile window starts."""
    nq_real = q.shape[0]
    nt = n_easy + n_hard
    slots = nt * 128
    e_slots, h_slots = n_easy * 128, n_hard * 128

    qk = _hkeys(q.astype(np.float32), glo, ghi)
    sk = _hkeys(ss.astype(np.float32), glo, ghi)
    oq = np.argsort(qk, kind="stable")
    # ss must already be sorted by its key for searchsorted windows
    qs = q[oq]
    pos = np.searchsorted(np.sort(sk), qk[oq])
    ncand = min(48, s_total)
    start = (pos - ncand // 2).clip(0, s_total - ncand)
    cand = start[:, None] + np.arange(ncand)[None, :]
    dc = ((qs[:, None, :] - ss[cand]) ** 2).sum(-1)
    r3sq = np.sort(dc, 1)[:, 2] * (1 + 1e-5) + 2e-6
    # window per query from full distance matrix (fp32 blas, with margin)
    qf, sf = qs.astype(np.float32), ss.astype(np.float32)
    D = ((qf ** 2).sum(-1)[:, None] + (sf ** 2).sum(-1)[None, :]
         - 2.0 * (qf @ sf.T)).astype(np.float64)
    within = D <= (r3sq[:, None] + 4e-6)
    first = np.argmax(within, 1)
    last = s_total - 1 - np.argmax(within[:, ::-1], 1)
    wq = last - first + 1

    easy = [i for i in range(nq_real) if wq[i] <= w_easy]
    hard = [i for i in range(nq_real) if wq[i] > w_easy]

    def greedy(lst):
        """Pack hilbert-ordered queries into tiles with union <= w_easy."""
        tiles, cur, lo, hi = [], [], None, None
        for i in lst:
            nlo = first[i] if lo is None else min(lo, first[i])
            nhi = last[i] if hi is None else max(hi, last[i])
            if len(cur) < 128 and nhi - nlo + 1 <= w_easy:
                cur.append(i)
                lo, hi = nlo, nhi
            else:
                tiles.append(cur)
                cur, lo, hi = [i], first[i], last[i]
        if cur:
            tiles.append(cur)
        return tiles

    for _ in range(400):
        if len(easy) > e_slots:
            drop = set(sorted(easy, key=lambda i: wq[i])[e_slots:])
            hard += [i for i in easy if i in drop]
            easy = [i for i in easy if i not in drop]
        tiles = greedy(easy)
        if len(tiles) <= n_easy:
            break
        drop = set(sorted(easy, key=lambda i: wq[i])[-8:])
        hard += [i for i in easy if i in drop]
        easy = [i for i in easy if i not in drop]
    if len(hard) > h_slots:
        # degraded fallback (windows of forced-back queries get clamped);
        # should not trigger on sane inputs
        back = set(sorted(hard, key=lambda i: wq[i])[:len(hard) - h_slots])
        easy = sorted(easy + [i for i in hard if i in back],
                      key=lambda i: int(np.searchsorted(np.sort(qk), qk[oq[i]])))
        hard = [i for i in hard if i not in back]
        tiles = greedy(easy)[:n_easy]
        placed = set()
        for tl in tiles:
            placed |= set(tl)
        hard += [i for i in easy if i not in placed]
        easy = [i for i in easy if i in placed]
        hard = hard[:h_slots]

    def pad128(tl):
        # cyclic duplication spreads pad weight evenly over the tile's
        # members (pads are counted in the BN-stat population)
        base = list(tl)
        j = 0
        while len(tl) < 128:
            tl.append(base[j % len(base)])
            j += 1
        return tl

    tiles = [pad128(tl) for tl in tiles]
    while len(tiles) < n_easy:
        tiles.append(list(tiles[-1]))
    a_t = []
    for tl in tiles:
        lo = min(first[i] for i in tl)
        a_t.append(int(min(lo, s_total - w_easy)))

    hard = sorted(hard, key=lambda i: wq[i])
    while len(hard) < h_slots:
        hard.append((len(hard) * 97 + 13) % nq_real)

    flat_easy = [i for tl in tiles for i in tl]
    perm = np.array([oq[i] for i in flat_easy] + [oq[i] for i in hard])
    assert perm.shape[0] == slots
    return perm, a_t


_PROGRAM_CACHE = {}


def make_core_inputs(sampled_xyz, sampled_features, original_xyz,
                     original_features, w0, w1, g0, bt0, g1, bt1, core,
                     n_easy=N_EASY, n_hard=N_HARD, w_easy=W_EASY):
    b, h = core // 2, core % 2
    s_total = sampled_xyz.shape[1]
    nq_real = original_xyz.shape[1] // 2
    nt = n_easy + n_hard
    nq = nt * 128
    f32 = np.float32

    s64 = np.asarray(sampled_xyz[b], np.float64)
    allp = np.vstack([np.asarray(original_xyz[b], f32),
                      np.asarray(sampled_xyz[b], f32)])
    glo, ghi = allp.min(0), allp.max(0)
    os_ = np.argsort(_hkeys(s64.astype(f32), glo, ghi), kind="stable")
    ss = s64[os_]
    sfeat = np.ascontiguousarray(
        np.asarray(sampled_features[b], f32)[os_])

    q = np.asarray(original_xyz[b, h * nq_real:(h + 1) * nq_real], np.float64)
    of = np.asarray(original_features[b, h * nq_real:(h + 1) * nq_real], f32)

    perm, a_t = _pack_core(q, ss, glo, ghi, w_easy, n_easy, n_hard, s_total)
    a_t = list(a_t) + [0] * n_hard

    lhsT = np.zeros((KROWS, nt, 128), ml_dtypes.bfloat16)
    rhsE = np.zeros((KROWS, max(n_easy, 1), w_easy), ml_dtypes.bfloat16)
    rhsH = np.zeros((KROWS, max(n_hard, 1), s_total), ml_dtypes.bfloat16)
    xn2p = np.zeros((128, nt), f32)  # expanded to xn23 below
    for t in range(nt):
        ql = perm[t * 128:(t + 1) * 128]
        qt = q[ql]
        c = qt.mean(0)
        u = qt - c
        if t < n_easy:
            v = ss[a_t[t]:a_t[t] + w_easy] - c
            L, R = _score_rows(u, v)
            rhsE[:, t, :] = R
        else:
            v = ss - c
            L, R = _score_rows(u, v)
            rhsH[:, t - n_easy, :] = R
        lhsT[:, t, :] = L
        xn2p[:, t] = (u * u).sum(-1).astype(f32)

    of_perm = of[perm]                                   # [nq, CO]
    af = np.repeat(np.asarray(a_t, f32)[None, :], 128, 0)
    return {
        "lhsT": lhsT,
        "rhsE": rhsE,
        "rhsH": rhsH,
        "xn23": np.ascontiguousarray(
            np.repeat(xn2p[:, :, None], 3, axis=2)),
        "af3": np.ascontiguousarray(
            np.repeat(af[:, :, None], 3, axis=2)),
        "sfeat": sfeat,
        "ofT": _bf16(np.ascontiguousarray(of_perm.T)),
        "w0T": _bf16(np.ascontiguousarray(
            w0.T.reshape(3, 128, C1).transpose(1, 0, 2))),
        "w1T": _bf16(np.ascontiguousarray(
            w1.T.reshape(2, 128, C2).transpose(1, 0, 2))),
        "g0p": np.ascontiguousarray(g0.reshape(2, 128).T).astype(f32),
        "bt0p": np.ascontiguousarray(bt0.reshape(2, 128).T).astype(f32),
        "g1p": np.ascontiguousarray(g1.reshape(2, 128).T).astype(f32),
        "bt1p": np.ascontiguousarray(bt1.reshape(2, 128).T).astype(f32),
        "eye": np.eye(128, dtype=f32),
    }, perm


def kernel(sampled_xyz, sampled_features, original_xyz, original_features,
           w0, b0, g0, bt0, w1, b1, g1, bt1, k):
    assert int(k) == 3
    from concourse.bass_utils import run_bass_kernel_spmd

    key = "full"
    if key not in _PROGRAM_CACHE:
        _PROGRAM_CACHE[key] = build_program()
    nc = _PROGRAM_CACHE[key]

    args = (sampled_xyz, sampled_features, original_xyz, original_features,
            w0, w1, g0, bt0, g1, bt1)
    args = [np.asarray(a, np.float32) for a in args]
    in_maps, perms = [], []
    for c in range(NCORES):
        im, perm = make_core_inputs(*args, core=c)
        in_maps.append(im)
        perms.append(perm)
    res = run_bass_kernel_spmd(nc, in_maps, core_ids=list(range(NCORES)))
    out = np.empty((B, N, C2), np.float32)
    nq = NT * 128
    for c in range(NCORES):
        b, h = c // 2, c % 2
        yT = res.results[c]["yT"]            # [2, 128, nq]
        y = yT.reshape(256, nq).T            # [nq, 256]
        inv = np.zeros(QP, np.int64)
        inv[perms[c][::-1]] = np.arange(nq)[::-1]  # first occurrence wins
        out[b, h * QP:(h + 1) * QP] = y[inv]
    return out


# revision 36
# speedup vs baseline: 1.1815x; 1.0268x over previous
"""PointNet++ FeaturePropagation Trainium2 kernel (8-core SPMD).

Per core c of 8: batch b = c//2, query-half h = c%2 (8192 original points).
KNN scores are computed on the PE as a 27-row bf16-triple decomposition of
2*(q-c_t).(s-c_t) - |s-c_t|^2 per query tile (center c_t), which makes every
product exact and keeps fp32 PSUM accumulation error ~1e-7 -- required
because 3rd/4th-neighbor distance gaps go down to ~1e-7 on this data.
Queries are Hilbert-sorted on the host; each 128-query tile scores against
a contiguous window of W=512 Hilbert-sorted samples (provably containing
the true 3-NN via a cheap host-side 3rd-NN upper bound); outlier queries
that need wider windows go to 3 dense tiles scored against all 4096
samples.  DVE max/max_index extract top-8; inverse-distance weights and
the gather/interp/MLP/BatchNorm pipeline follow (BN stats via 8-core
AllReduce; conv biases cancel through BN and are skipped).
Host does layout transforms (sorting/transpose/decomposition) and the
final unshard/unpermute.
"""

import numpy as np
import ml_dtypes

import concourse.bass as bass
import concourse.bacc as bacc
import concourse.mybir as mybir
import concourse.tile as tile

F32 = mybir.dt.float32
BF16 = mybir.dt.bfloat16
U32 = mybir.dt.uint32
ALU = mybir.AluOpType
ACT = mybir.ActivationFunctionType
AX = mybir.AxisListType

B, S, N = 4, 4096, 16384
CS, CO = 256, 128
C1, C2 = 256, 256
NCORES = 8
QP = N // 2          # real queries per core
N_EASY, N_HARD, W_EASY = 62, 3, 512
NT = N_EASY + N_HARD  # 65 tiles -> 8320 slots (128 duplicate pads)
BN_EPS = 1e-5
W_EPS = 1e-8
KROWS = 27


def divisor_csz(nq):
    for d in range(512, 0, -1):
        if nq % d == 0:
            return d


def build_program(n_easy=N_EASY, n_hard=N_HARD, w_easy=W_EASY, s_total=S,
                  n_cores=NCORES, n_points_total=None):
    nt = n_easy + n_hard
    nq = nt * 128
    if n_points_total is None:
        n_points_total = n_cores * nq
    nc = bacc.Bacc("TRN2", target_bir_lowering=False, debug=False,
                   num_devices=n_cores)

    d_lhsT = nc.dram_tensor("lhsT", [KROWS, nt, 128], BF16,
                            kind="ExternalInput")
    d_rhsE = nc.dram_tensor("rhsE", [KROWS, max(n_easy, 1), w_easy], BF16,
                            kind="ExternalInput")
    d_rhsH = nc.dram_tensor("rhsH", [KROWS, max(n_hard, 1), s_total], BF16,
                            kind="ExternalInput")
    d_xn23 = nc.dram_tensor("xn23", [128, nt, 3], F32, kind="ExternalInput")
    d_af3 = nc.dram_tensor("af3", [128, nt, 3], F32, kind="ExternalInput")
    d_sfeat = nc.dram_tensor("sfeat", [s_total, CS], F32,
                             kind="ExternalInput")
    d_ofT = nc.dram_tensor("ofT", [CO, nq], BF16, kind="ExternalInput")
    d_w0T = nc.dram_tensor("w0T", [128, 3, C1], BF16, kind="ExternalInput")
    d_w1T = nc.dram_tensor("w1T", [128, 2, C2], BF16, kind="ExternalInput")
    d_g0 = nc.dram_tensor("g0p", [128, 2], F32, kind="ExternalInput")
    d_bt0 = nc.dram_tensor("bt0p", [128, 2], F32, kind="ExternalInput")
    d_g1 = nc.dram_tensor("g1p", [128, 2], F32, kind="ExternalInput")
    d_bt1 = nc.dram_tensor("bt1p", [128, 2], F32, kind="ExternalInput")
    d_eye = nc.dram_tensor("eye", [128, 128], F32, kind="ExternalInput")
    d_out = nc.dram_tensor("yT", [2, 128, nq], F32, kind="ExternalOutput")
    d_dbgi = nc.dram_tensor("dbgi", [128, nt, 3], F32, kind="ExternalOutput")
    d_dbgw = nc.dram_tensor("dbgw", [128, nt, 3], F32, kind="ExternalOutput")

    gs = 2
    with tile.TileContext(nc) as tc:
        with (
            tc.tile_pool(name="const", bufs=1) as cpool,
            tc.tile_pool(name="big", bufs=1) as bigp,
            tc.tile_pool(name="sc_sbE", bufs=2) as scpE,
            tc.tile_pool(name="sc_sbH", bufs=1) as scpH,
            tc.tile_pool(name="small", bufs=3) as smp,
            tc.tile_pool(name="gath", bufs=2) as gathp,
            tc.tile_pool(name="ps_sc", bufs=4, space="PSUM") as ps_sc,
            tc.tile_pool(name="ps_sm", bufs=2, space="PSUM") as ps_sm,
            tc.tile_pool(name="ps_mm1", bufs=2, space="PSUM") as ps_mm1,
            tc.tile_pool(name="dram", bufs=1, space="DRAM") as dramp,
        ):
            def load(pool, name, dram, shape, dt=F32):
                t_ = pool.tile(shape, dt, tag=name)
                nc.sync.dma_start(t_[:], dram[:])
                return t_

            eye = load(cpool, "eye", d_eye, [128, 128])
            w0T = load(cpool, "w0T", d_w0T, [128, 3, C1], BF16)
            w1T = load(cpool, "w1T", d_w1T, [128, 2, C2], BF16)
            g0p = load(cpool, "g0p", d_g0, [128, 2])
            bt0p = load(cpool, "bt0p", d_bt0, [128, 2])
            g1p = load(cpool, "g1p", d_g1, [128, 2])
            bt1p = load(cpool, "bt1p", d_bt1, [128, 2])
            xn23 = load(cpool, "xn23", d_xn23, [128, nt, 3])
            af3 = load(cpool, "af3", d_af3, [128, nt, 3])
            lhsT = load(cpool, "lhsT", d_lhsT, [KROWS, nt, 128], BF16)

            y0a = bigp.tile([128, nq], F32, tag="y0a")
            y0b = bigp.tile([128, nq], F32, tag="y0b")
            idx_all = bigp.tile([128, nt, 3], F32, tag="idx_all")
            w_all = bigp.tile([128, nt, 3], F32, tag="w_all")
            v8a = bigp.tile([128, nt, 8], F32, tag="v8a")
            i8a = bigp.tile([128, nt, 8], U32, tag="i8a")

            # ---------------- KNN selection (in blocks for overlap) -------
            # per-block weight math + wrapped-index build lets the dataflow
            # scheduler start block 0's gather/interp/MLP0 while later
            # blocks' KNN scans are still running
            i8f = bigp.tile([128, nt, 3], F32, tag="i8f")
            d3a = bigp.tile([128, nt, 3], F32, tag="d3a")
            r3a = bigp.tile([128, nt, 3], F32, tag="r3a")
            rsa = bigp.tile([128, nt], F32, tag="rsa")
            idx16 = bigp.tile([128, nt, 3], mybir.dt.int16, tag="idx16")
            ncols = nt * 24
            wrapped = bigp.tile([128, ncols], mybir.dt.int16, tag="wrapped")
            BLK = 16
            for t0 in range(0, nt, BLK):
                t1 = min(t0 + BLK, nt)
                for t in range(t0, t1):
                    if t < n_easy:
                        wt = w_easy
                        rt = scpE.tile([KROWS, w_easy], BF16, tag="rhsEt")
                        nc.sync.dma_start(rt[:], d_rhsE[:, t, :])
                        ps = ps_sc.tile([128, 512], F32, tag="ps_score")
                        nc.tensor.matmul(ps[:, 0:wt], lhsT[:, t, :],
                                         rt[:], start=True, stop=True)
                        sc = scpE.tile([128, w_easy], F32, tag="scoresE")
                        nc.scalar.copy(sc[:], ps[:, 0:wt])
                    else:
                        wt = s_total
                        j = t - n_easy
                        rt = scpH.tile([KROWS, s_total], BF16, tag="rhsHt")
                        nc.sync.dma_start(rt[:], d_rhsH[:, j, :])
                        sc = scpH.tile([128, s_total], F32, tag="scoresH")
                        for q8 in range(s_total // 512):
                            ps = ps_sc.tile([128, 512], F32, tag="ps_score")
                            col = q8 * 512
                            nc.tensor.matmul(ps[:], lhsT[:, t, :],
                                             rt[:, col:col + 512],
                                             start=True, stop=True)
                            nc.scalar.copy(sc[:, col:col + 512], ps[:])

                    nc.vector.max(v8a[:, t, :], sc[:, 0:wt])
                    nc.vector.max_index(i8a[:, t, :], v8a[:, t, :],
                                        sc[:, 0:wt])

                # batched weights + index remap for this block
                bs = slice(t0, t1)
                nc.vector.tensor_copy(i8f[:, bs, :], i8a[:, bs, 0:3])
                nc.vector.tensor_tensor(out=idx_all[:, bs, :],
                                        in0=i8f[:, bs, :],
                                        in1=af3[:, bs, :], op=ALU.add)
                nc.vector.tensor_tensor(out=d3a[:, bs, :], in0=xn23[:, bs, :],
                                        in1=v8a[:, bs, 0:3], op=ALU.subtract)
                nc.vector.tensor_scalar_add(d3a[:, bs, :], d3a[:, bs, :],
                                            W_EPS)
                nc.vector.reciprocal(r3a[:, bs, :], d3a[:, bs, :])
                nc.vector.tensor_reduce(out=rsa[:, bs], in_=r3a[:, bs, :],
                                        axis=AX.X, op=ALU.add)
                nc.vector.reciprocal(rsa[:, bs], rsa[:, bs])
                for kk in range(3):
                    nc.vector.tensor_tensor(out=w_all[:, bs, kk],
                                            in0=r3a[:, bs, kk],
                                            in1=rsa[:, bs], op=ALU.mult)
                # 16-partition-wrapped int16 indices for dma_gather: flat
                # order i = t*384 + k*128 + q -> partition q%16, col
                # t*24 + k*8 + q//16 -> 8 strided DMAs per block
                nc.vector.tensor_copy(idx16[:, bs, :], idx_all[:, bs, :])
                for u in range(8):
                    nc.gpsimd.dma_start(
                        wrapped[0:16, t0 * 24 + u:t1 * 24:8].rearrange(
                            "p (t k) -> p t k", k=3),
                        idx16[16 * u:16 * (u + 1), bs, :])
                for rep in range(1, 8):
                    nc.gpsimd.dma_start(
                        wrapped[16 * rep:16 * (rep + 1), t0 * 24:t1 * 24],
                        wrapped[0:16, t0 * 24:t1 * 24])

            nc.sync.dma_start(d_dbgi[:], idx_all[:])
            nc.sync.dma_start(d_dbgw[:], w_all[:])

            # ---------------- gather + interp + matmul0 ----------------
            n_g = (nt + gs - 1) // gs
            for g in range(n_g):
                t0 = g * gs
                gt = min(gs, nt - t0)
                gbuf = gathp.tile([128, gs * 3, CS], F32, tag="gath")
                ofTg = smp.tile([CO, gs * 128], BF16, tag="ofTg")
                nc.sync.dma_start(ofTg[:, 0:gt * 128],
                                  d_ofT[:, t0 * 128:(t0 + gt) * 128])
                nc.gpsimd.dma_gather(
                    out_ap=gbuf[:, 0:gt * 3, :],
                    in_ap=d_sfeat[:],
                    idxs_ap=wrapped[:, t0 * 24:(t0 + gt) * 24],
                    num_idxs=gt * 384,
                    num_idxs_reg=gt * 384,
                    elem_size=CS,
                )
                for tt in range(gt):
                    t = t0 + tt
                    interp = smp.tile([128, CS], F32, tag="interp")
                    acc = smp.tile([128, CS], F32, tag="interp_acc")
                    nc.vector.tensor_scalar(
                        out=acc[:], in0=gbuf[:, tt * 3, :],
                        scalar1=w_all[:, t, 0:1], scalar2=None, op0=ALU.mult)
                    nc.vector.scalar_tensor_tensor(
                        out=interp[:], in0=gbuf[:, tt * 3 + 1, :],
                        scalar=w_all[:, t, 1:2], in1=acc[:],
                        op0=ALU.mult, op1=ALU.add)
                    nc.vector.scalar_tensor_tensor(
                        out=interp[:], in0=gbuf[:, tt * 3 + 2, :],
                        scalar=w_all[:, t, 2:3], in1=interp[:],
                        op0=ALU.mult, op1=ALU.add)
                    iT = smp.tile([128, 2, 128], BF16, tag="interpT")
                    for hh in range(2):
                        ps_tr = ps_sm.tile([128, 128], F32, tag="ps_small")
                        nc.tensor.transpose(
                            ps_tr[:], interp[:, hh * 128:(hh + 1) * 128],
                            eye[:])
                        nc.scalar.copy(iT[:, hh, :], ps_tr[:])
                    for m, ybuf in ((0, y0a), (1, y0b)):
                        ps_y = ps_sm.tile([128, 128], F32, tag="ps_small")
                        mcol = slice(m * 128, (m + 1) * 128)
                        nc.tensor.matmul(ps_y[:], w0T[:, 0, mcol],
                                         ofTg[:, tt * 128:(tt + 1) * 128],
                                         start=True, stop=False)
                        nc.tensor.matmul(ps_y[:], w0T[:, 1, mcol],
                                         iT[:, 0, :], start=False, stop=False)
                        nc.tensor.matmul(ps_y[:], w0T[:, 2, mcol],
                                         iT[:, 1, :], start=False, stop=True)
                        nc.scalar.copy(ybuf[:, t * 128:(t + 1) * 128], ps_y[:])

            # ---------------- BN helpers ----------------
            csz = divisor_csz(nq)

            def bn_allreduce(ya, yb, gp, btp, tag):
                """Returns (a, bhat) [128,2] with yhat = Relu(y*a + bhat)."""
                nchunk = 8
                csz_s = nq // nchunk
                sump = smp.tile([128, 2 * nchunk], F32, tag=f"sump{tag}")
                sqp = smp.tile([128, 2 * nchunk], F32, tag=f"sqp{tag}")
                scratch = cpool.tile([128, csz_s], F32, tag="bn_scratch")
                for m, ybuf in ((0, ya), (1, yb)):
                    for ch in range(nchunk):
                        sl = slice(ch * csz_s, (ch + 1) * csz_s)
                        col = m * nchunk + ch
                        nc.vector.tensor_reduce(
                            out=sump[:, col:col + 1], in_=ybuf[:, sl],
                            axis=AX.X, op=ALU.add)
                        nc.scalar.activation(
                            out=scratch[:], in_=ybuf[:, sl], func=ACT.Square,
                            accum_out=sqp[:, col:col + 1])
                stats = smp.tile([128, 4], F32, tag=f"stats{tag}")
                nc.vector.tensor_reduce(
                    out=stats[:, 0:2],
                    in_=sump[:].rearrange("p (m c) -> p m c", m=2),
                    axis=AX.X, op=ALU.add)
                nc.vector.tensor_reduce(
                    out=stats[:, 2:4],
                    in_=sqp[:].rearrange("p (m c) -> p m c", m=2),
                    axis=AX.X, op=ALU.add)
                bi = dramp.tile([128, 4], F32, tag=f"bi{tag}")
                bo = dramp.tile([128, 4], F32, tag=f"bo{tag}")
                nc.gpsimd.dma_start(bi[:], stats[:])
                nc.gpsimd.collective_compute(
                    "AllReduce", ALU.add,
                    replica_groups=[list(range(n_cores))],
                    ins=[bi.opt()], outs=[bo.opt()])
                gstats = smp.tile([128, 4], F32, tag=f"gstats{tag}")
                nc.gpsimd.dma_start(gstats[:], bo[:])

                mean = smp.tile([128, 2], F32, tag=f"mean{tag}")
                nc.vector.tensor_scalar_mul(mean[:], gstats[:, 0:2],
                                            1.0 / n_points_total)
                vpe = smp.tile([128, 2], F32, tag=f"vpe{tag}")
                nc.vector.tensor_scalar_mul(vpe[:], gstats[:, 2:4],
                                            1.0 / n_points_total)
                msq = smp.tile([128, 2], F32, tag=f"msq{tag}")
                nc.vector.tensor_tensor(out=msq[:], in0=mean[:], in1=mean[:],
                                        op=ALU.mult)
                nc.vector.tensor_tensor(out=vpe[:], in0=vpe[:], in1=msq[:],
                                        op=ALU.subtract)
                nc.vector.tensor_scalar_add(vpe[:], vpe[:], BN_EPS)
                rcp = smp.tile([128, 2], F32, tag=f"rcp{tag}")
                nc.vector.reciprocal(rcp[:], vpe[:])
                rsq = smp.tile([128, 2], F32, tag=f"rsq{tag}")
                nc.scalar.activation(out=rsq[:], in_=rcp[:], func=ACT.Sqrt)
                t1 = smp.tile([128, 2], F32, tag=f"t1{tag}")
                nc.vector.tensor_tensor(out=t1[:], in0=rsq[:], in1=rsq[:],
                                        op=ALU.mult)
                nc.vector.tensor_tensor(out=t1[:], in0=t1[:], in1=vpe[:],
                                        op=ALU.mult)
                nc.vector.tensor_scalar(out=t1[:], in0=t1[:], scalar1=-0.5,
                                        scalar2=1.5, op0=ALU.mult, op1=ALU.add)
                nc.vector.tensor_tensor(out=rsq[:], in0=rsq[:], in1=t1[:],
                                        op=ALU.mult)
                a = smp.tile([128, 2], F32, tag=f"a{tag}")
                nc.vector.tensor_tensor(out=a[:], in0=gp[:], in1=rsq[:],
                                        op=ALU.mult)
                bhat = smp.tile([128, 2], F32, tag=f"bhat{tag}")
                nc.vector.tensor_tensor(out=bhat[:], in0=mean[:], in1=a[:],
                                        op=ALU.mult)
                nc.vector.tensor_tensor(out=bhat[:], in0=btp[:], in1=bhat[:],
                                        op=ALU.subtract)
                return a, bhat

            # ---------------- layer 1 ----------------
            a0, b0h = bn_allreduce(y0a, y0b, g0p, bt0p, "0")
            for ch in range(nq // csz):
                sl = slice(ch * csz, (ch + 1) * csz)
                yh0 = smp.tile([128, csz], BF16, tag="yh0")
                yh1 = smp.tile([128, csz], BF16, tag="yh1")
                nc.scalar.activation(out=yh0[:], in_=y0a[:, sl], func=ACT.Relu,
                                     scale=a0[:, 0:1], bias=b0h[:, 0:1])
                nc.scalar.activation(out=yh1[:], in_=y0b[:, sl], func=ACT.Relu,
                                     scale=a0[:, 1:2], bias=b0h[:, 1:2])
                for m, ybuf in ((0, y0a), (1, y0b)):
                    ps1 = ps_mm1.tile([128, csz], F32, tag="ps_mm1")
                    mcol = slice(m * 128, (m + 1) * 128)
                    nc.tensor.matmul(ps1[:], w1T[:, 0, mcol], yh0[:],
                                     start=True, stop=False)
                    nc.tensor.matmul(ps1[:], w1T[:, 1, mcol], yh1[:],
                                     start=False, stop=True)
                    nc.scalar.copy(ybuf[:, sl], ps1[:])

            # ---------------- layer 2 BN + out ----------------
            a1, b1h = bn_allreduce(y0a, y0b, g1p, bt1p, "1")
            for ch in range(nq // csz):
                sl = slice(ch * csz, (ch + 1) * csz)
                for m, ybuf in ((0, y0a), (1, y0b)):
                    o = smp.tile([128, csz], F32, tag="outsb")
                    nc.scalar.activation(out=o[:], in_=ybuf[:, sl],
                                         func=ACT.Relu,
                                         scale=a1[:, m:m + 1],
                                         bias=b1h[:, m:m + 1])
                    nc.sync.dma_start(d_out[m, :, sl], o[:])

    nc.compile()
    return nc


# ======================= host-side preparation =======================

def _hilbert_d3(x, y, z, order=10):
    X = np.stack([x, y, z], axis=0).astype(np.uint32).copy()
    M = np.uint32(1 << (order - 1))
    Q = M
    while Q > 1:
        P = np.uint32(Q - 1)
        for i in range(3):
            cond = (X[i] & Q) != 0
            X[0] = np.where(cond, X[0] ^ P, X[0])
            t = (X[0] ^ X[i]) & P
            X[0] ^= t
            X[i] ^= t
        Q >>= 1
    for i in range(1, 3):
        X[i] ^= X[i - 1]
    t2 = np.zeros_like(X[0])
    Q = M
    while Q > 1:
        t2 = np.where((X[2] & Q) != 0, t2 ^ np.uint32(Q - 1), t2)
        Q >>= 1
    for i in range(3):
        X[i] ^= t2
    key = np.zeros(X.shape[1], dtype=np.uint64)
    for b in range(order - 1, -1, -1):
        for i in range(3):
            key = ((key << np.uint64(1))
                   | ((X[i] >> np.uint32(b)) & np.uint32(1)).astype(np.uint64))
    return key


def _hkeys(pts, lo, hi, order=10):
    qq = ((pts - lo) / np.maximum(hi - lo, 1e-9)).clip(0.0, 1.0)
    qq = (qq * ((1 << order) - 1)).astype(np.uint32)
    return _hilbert_d3(qq[:, 0], qq[:, 1], qq[:, 2], order)


def _bf16(x):
    return np.asarray(np.asarray(x, np.float32), ml_dtypes.bfloat16)


def _dec3(x):
    """fp64 -> three bf16 terms (residual ~2^-24 |x|)."""
    h = _bf16(x)
    r = x - np.asarray(h, np.float64)
    m = _bf16(r)
    r2 = r - np.asarray(m, np.float64)
    return h, m, _bf16(r2)


def _score_rows(u, v):
    """u: [nq, 3] fp64 query offsets; v: [w, 3] fp64 sample offsets.
    Returns L [27, nq] bf16, R [27, w] bf16 with sum_k L[k] outer R[k]
    ~= 2 u.v - |v|^2 (error ~1e-7 for |u|,|v| <~ 1)."""
    uh, um, ul = _dec3(2.0 * u)
    vh, vm, vl = _dec3(v)
    n = v * v
    n1, n2, n3 = _dec3(-n)
    one = np.ones(u.shape[0], ml_dtypes.bfloat16)
    L, R = [], []
    for ci in range(3):       # hi block: products ~O(r^2), cancel early
        L += [uh[:, ci], one]
        R += [vh[:, ci], n1[:, ci]]
    for ci in range(3):       # med block
        L += [um[:, ci], uh[:, ci], one]
        R += [vh[:, ci], vm[:, ci], n2[:, ci]]
    for ci in range(3):       # lo block
        L += [ul[:, ci], uh[:, ci], um[:, ci], one]
        R += [vh[:, ci], vl[:, ci], vm[:, ci], n3[:, ci]]
    return (np.ascontiguousarray(np.stack(L)),
            np.ascontiguousarray(np.stack(R)))


def _pack_core(q, ss, glo, ghi, w_easy, n_easy, n_hard, s_total):
    """q: [nq_real, 3] fp64 queries (this core); ss: [s_total, 3] fp64
    samples sorted by Hilbert key on the (glo, ghi) grid.  Returns (perm,
    a_t) where perm is the slot -> local-query-index map (len nt*128, with
    duplicate pads) and a_t the per-easy-tile window starts."""
    nq_real = q.shape[0]
    nt = n_easy + n_hard
    slots = nt * 128
    e_slots, h_slots = n_easy * 128, n_hard * 128

    qk = _hkeys(q.astype(np.float32), glo, ghi)
    sk = _hkeys(ss.astype(np.float32), glo, ghi)
    oq = np.argsort(qk, kind="stable")
    # ss must already be sorted by its key for searchsorted windows
    qs = q[oq]
    pos = np.searchsorted(np.sort(sk), qk[oq])
    ncand = min(48, s_total)
    start = (pos - ncand // 2).clip(0, s_total - ncand)
    cand = start[:, None] + np.arange(ncand)[None, :]
    dc = ((qs[:, None, :] - ss[cand]) ** 2).sum(-1)
    r3sq = np.sort(dc, 1)[:, 2] * (1 + 1e-5) + 2e-6
    # window per query from full distance matrix (fp32 blas, with margin)
    qf, sf = qs.astype(np.float32), ss.astype(np.float32)
    D = ((qf ** 2).sum(-1)[:, None] + (sf ** 2).sum(-1)[None, :]
         - 2.0 * (qf @ sf.T)).astype(np.float64)
    within = D <= (r3sq[:, None] + 4e-6)
    first = np.argmax(within, 1)
    last = s_total - 1 - np.argmax(within[:, ::-1], 1)
    wq = last - first + 1

    easy = [i for i in range(nq_real) if wq[i] <= w_easy]
    hard = [i for i in range(nq_real) if wq[i] > w_easy]

    def greedy(lst):
        """Pack hilbert-ordered queries into tiles with union <= w_easy."""
        tiles, cur, lo, hi = [], [], None, None
        for i in lst:
            nlo = first[i] if lo is None else min(lo, first[i])
            nhi = last[i] if hi is None else max(hi, last[i])
            if len(cur) < 128 and nhi - nlo + 1 <= w_easy:
                cur.append(i)
                lo, hi = nlo, nhi
            else:
                tiles.append(cur)
                cur, lo, hi = [i], first[i], last[i]
        if cur:
            tiles.append(cur)
        return tiles

    for _ in range(400):
        if len(easy) > e_slots:
            drop = set(sorted(easy, key=lambda i: wq[i])[e_slots:])
            hard += [i for i in easy if i in drop]
            easy = [i for i in easy if i not in drop]
        tiles = greedy(easy)
        if len(tiles) <= n_easy:
            break
        drop = set(sorted(easy, key=lambda i: wq[i])[-8:])
        hard += [i for i in easy if i in drop]
        easy = [i for i in easy if i not in drop]
    if len(hard) > h_slots:
        # degraded fallback (windows of forced-back queries get clamped);
        # should not trigger on sane inputs
        back = set(sorted(hard, key=lambda i: wq[i])[:len(hard) - h_slots])
        easy = sorted(easy + [i for i in hard if i in back],
                      key=lambda i: int(np.searchsorted(np.sort(qk), qk[oq[i]])))
        hard = [i for i in hard if i not in back]
        tiles = greedy(easy)[:n_easy]
        placed = set()
        for tl in tiles:
            placed |= set(tl)
        hard += [i for i in easy if i not in placed]
        easy = [i for i in easy if i in placed]
        hard = hard[:h_slots]

    def pad128(tl):
        # cyclic duplication spreads pad weight evenly over the tile's
        # members (pads are counted in the BN-stat population)
        base = list(tl)
        j = 0
        while len(tl) < 128:
            tl.append(base[j % len(base)])
            j += 1
        return tl

    tiles = [pad128(tl) for tl in tiles]
    while len(tiles) < n_easy:
        tiles.append(list(tiles[-1]))
    a_t = []
    for tl in tiles:
        lo = min(first[i] for i in tl)
        a_t.append(int(min(lo, s_total - w_easy)))

    hard = sorted(hard, key=lambda i: wq[i])
    while len(hard) < h_slots:
        hard.append((len(hard) * 97 + 13) % nq_real)

    flat_easy = [i for tl in tiles for i in tl]
    perm = np.array([oq[i] for i in flat_easy] + [oq[i] for i in hard])
    assert perm.shape[0] == slots
    return perm, a_t


_PROGRAM_CACHE = {}


def make_core_inputs(sampled_xyz, sampled_features, original_xyz,
                     original_features, w0, w1, g0, bt0, g1, bt1, core,
                     n_easy=N_EASY, n_hard=N_HARD, w_easy=W_EASY):
    b, h = core // 2, core % 2
    s_total = sampled_xyz.shape[1]
    nq_real = original_xyz.shape[1] // 2
    nt = n_easy + n_hard
    nq = nt * 128
    f32 = np.float32

    s64 = np.asarray(sampled_xyz[b], np.float64)
    allp = np.vstack([np.asarray(original_xyz[b], f32),
                      np.asarray(sampled_xyz[b], f32)])
    glo, ghi = allp.min(0), allp.max(0)
    os_ = np.argsort(_hkeys(s64.astype(f32), glo, ghi), kind="stable")
    ss = s64[os_]
    sfeat = np.ascontiguousarray(
        np.asarray(sampled_features[b], f32)[os_])

    q = np.asarray(original_xyz[b, h * nq_real:(h + 1) * nq_real], np.float64)
    of = np.asarray(original_features[b, h * nq_real:(h + 1) * nq_real], f32)

    perm, a_t = _pack_core(q, ss, glo, ghi, w_easy, n_easy, n_hard, s_total)
    a_t = list(a_t) + [0] * n_hard

    lhsT = np.zeros((KROWS, nt, 128), ml_dtypes.bfloat16)
    rhsE = np.zeros((KROWS, max(n_easy, 1), w_easy), ml_dtypes.bfloat16)
    rhsH = np.zeros((KROWS, max(n_hard, 1), s_total), ml_dtypes.bfloat16)
    xn2p = np.zeros((128, nt), f32)  # expanded to xn23 below
    for t in range(nt):
        ql = perm[t * 128:(t + 1) * 128]
        qt = q[ql]
        c = qt.mean(0)
        u = qt - c
        if t < n_easy:
            v = ss[a_t[t]:a_t[t] + w_easy] - c
            L, R = _score_rows(u, v)
            rhsE[:, t, :] = R
        else:
            v = ss - c
            L, R = _score_rows(u, v)
            rhsH[:, t - n_easy, :] = R
        lhsT[:, t, :] = L
        xn2p[:, t] = (u * u).sum(-1).astype(f32)

    of_perm = of[perm]                                   # [nq, CO]
    af = np.repeat(np.asarray(a_t, f32)[None, :], 128, 0)
    return {
        "lhsT": lhsT,
        "rhsE": rhsE,
        "rhsH": rhsH,
        "xn23": np.ascontiguousarray(
            np.repeat(xn2p[:, :, None], 3, axis=2)),
        "af3": np.ascontiguousarray(
            np.repeat(af[:, :, None], 3, axis=2)),
        "sfeat": sfeat,
        "ofT": _bf16(np.ascontiguousarray(of_perm.T)),
        "w0T": _bf16(np.ascontiguousarray(
            w0.T.reshape(3, 128, C1).transpose(1, 0, 2))),
        "w1T": _bf16(np.ascontiguousarray(
            w1.T.reshape(2, 128, C2).transpose(1, 0, 2))),
        "g0p": np.ascontiguousarray(g0.reshape(2, 128).T).astype(f32),
        "bt0p": np.ascontiguousarray(bt0.reshape(2, 128).T).astype(f32),
        "g1p": np.ascontiguousarray(g1.reshape(2, 128).T).astype(f32),
        "bt1p": np.ascontiguousarray(bt1.reshape(2, 128).T).astype(f32),
        "eye": np.eye(128, dtype=f32),
    }, perm


def kernel(sampled_xyz, sampled_features, original_xyz, original_features,
           w0, b0, g0, bt0, w1, b1, g1, bt1, k):
    assert int(k) == 3
    from concourse.bass_utils import run_bass_kernel_spmd

    key = "full"
    if key not in _PROGRAM_CACHE:
        _PROGRAM_CACHE[key] = build_program()
    nc = _PROGRAM_CACHE[key]

    args = (sampled_xyz, sampled_features, original_xyz, original_features,
            w0, w1, g0, bt0, g1, bt1)
    args = [np.asarray(a, np.float32) for a in args]
    in_maps, perms = [], []
    for c in range(NCORES):
        im, perm = make_core_inputs(*args, core=c)
        in_maps.append(im)
        perms.append(perm)
    res = run_bass_kernel_spmd(nc, in_maps, core_ids=list(range(NCORES)))
    out = np.empty((B, N, C2), np.float32)
    nq = NT * 128
    for c in range(NCORES):
        b, h = c // 2, c % 2
        yT = res.results[c]["yT"]            # [2, 128, nq]
        y = yT.reshape(256, nq).T            # [nq, 256]
        inv = np.zeros(QP, np.int64)
        inv[perms[c][::-1]] = np.arange(nq)[::-1]  # first occurrence wins
        out[b, h * QP:(h + 1) * QP] = y[inv]
    return out
